# revision 33
# baseline (speedup 1.0000x reference)
"""Causal multi-head attention on 8 TRN2 NeuronCores.

Problem: x[4, 2048, 768], 12 heads x d_head 64, causal softmax attention.

Sharding: core c handles batch b = c//2 and the 6-head group h0 = 6*(c%2).
Each core computes its partial output o_partial[2048, 768] = sum over its 6
heads of (softmax(QK^T/8) V) @ W_O.  The two cores sharing a batch are summed
on the host (part of unsharding), so the device graph needs no collectives.

Device layout (per core) avoids every on-chip transpose:
  - host passes x^T  [768, 2048]  (xt)
  - Q^T, K^T [384, 2048] = W^T @ x^T   (lhsT = W slice, rhs = x^T)
  - V        [2048, 384]  = x @ W_V    (lhsT = x^T slice, rhs = W_V)
  - scores^T blocks [128k, 512q] = (K^T slice).T @ Q^T slice  (per head, K=64)
  - exp on ACT (scale=1/8 folded in); causal mask = 0/1 multiply on diagonal
    blocks; V is stored augmented with a ones column per head so the single
    AV matmul produces both z^T rows (64) and the softmax denominators (row 64)
  - z^T normalized via DMA-broadcast reciprocal row, stored as ZT [384, 2048]
  - out = (ZT).T @ W_O   (lhsT = ZT slice, rhs = W_O)
All matmuls run in bf16 (1 cycle/column vs 4 for fp32; f32 PSUM accum).

Scheduling: the attention phase is ACT(exp)-bound (~35us of exp per head
pair vs ~25us of PE), while projections and the output projection are
PE-only.  So emission interleaves them: while pair hp's attention stream
keeps ACT busy, the PE also runs pair hp+1's Q/K projections (and, during
the last pair, the output-projection tiles), paced by a credit-fed global
queue with per-sweep deadlines (emission-order WAR safety) and eligibility
gates (fillers reserved for PE-thin phases; out-proj gated on the ZT writes
it reads being emitted).  q-supertiles are processed one at a time (not in
halves) so PSUM fits scores (4 banks) + z accumulators (2) + a
projection/output slot (2).  Every score block writes the two heads'
columns anchored at the psum bank boundary (par0 [q0,512) in bank 0, par1
[512,512+w) in bank 1): the concurrent matmuls hit different single-port
banks, and one exp instruction covers both heads.  Deferred AV / normalize
closures ride a global lag queue that flows across sweep and pair
boundaries; causal-mask multiplies run on the otherwise-idle GpSimd engine;
the last supertile's output projection is split so only the ZT[2] matmul
waits on the final normalize.

Note: the chip drops the PE from 2.4 to ~2.0 GHz (P0 power state) under
sustained load, so back-to-back benchmark runs measure ~10-13% slower than
cooled runs (~236us cooled, ~268us hot).
"""

import sys

if "/opt/trn_rl_repo" not in sys.path:
    sys.path.insert(0, "/opt/trn_rl_repo")

import numpy as np
import ml_dtypes

BF16NP = ml_dtypes.bfloat16


def _ensure_ntff_hook():
    """The agent image's `antenv` lacks `axon_hooks`, which bass_utils needs
    for trace=True under axon. Recreate it via sys.modules injection using the
    boot helper's ctypes wrapper around libaxon_pjrt.so."""
    import types
    if "antenv.axon_hooks" in sys.modules:
        return
    try:
        from trn_agent_boot.trn_boot import _ntff_profile_via_ctypes
        hook = _ntff_profile_via_ctypes("/opt/axon/libaxon_pjrt.so")
    except Exception:
        hook = None
    m = types.ModuleType("antenv.axon_hooks")
    m._hook = hook
    m.get_axon_ntff_profile_hook = lambda: m._hook
    def _set(h):
        m._hook = h
    m.set_axon_ntff_profile_hook = _set
    sys.modules["antenv.axon_hooks"] = m


_ensure_ntff_hook()

import concourse.bass as bass
import concourse.tile as tile
from concourse import bacc, mybir
from concourse.bass_utils import run_bass_kernel_spmd

F32 = mybir.dt.float32
BF16 = mybir.dt.bfloat16
AF = mybir.ActivationFunctionType

D = 768          # d_model
S = 2048         # seq
E = 64           # d_head
NHC = 6          # heads per core
HE = NHC * E     # 384
KD = D // 128    # 6 k-chunks over d_model
B = 4

AV_LAG = 8       # deferred-AV depth (keeps PE from stalling on exp)

LAST_EXEC_TIME_NS = None
_GRAPH_CACHE = {}


def _build_graph(qkv_bias: bool) -> bass.Bass:
    nc = bacc.Bacc("TRN2", target_bir_lowering=False)
    xt = nc.declare_dram_parameter("xt", [D, S], BF16, isOutput=False)
    wq = nc.declare_dram_parameter("wq", [D, HE], BF16, isOutput=False)
    wk = nc.declare_dram_parameter("wk", [D, HE], BF16, isOutput=False)
    wv = nc.declare_dram_parameter("wv", [D, HE], BF16, isOutput=False)
    wo = nc.declare_dram_parameter("wo", [HE, D], BF16, isOutput=False)
    mask = nc.declare_dram_parameter("mask", [128, 128], BF16, isOutput=False)
    if qkv_bias:
        bq = nc.declare_dram_parameter("bq", [HE, 1], F32, isOutput=False)
        bk = nc.declare_dram_parameter("bk", [HE, 1], F32, isOutput=False)
        bv = nc.declare_dram_parameter("bv", [1, HE], BF16, isOutput=False)
    out = nc.declare_dram_parameter("out", [S, D], F32, isOutput=True)

    with tile.TileContext(nc) as tc:
        with tc.tile_pool(name="persist", bufs=1) as persist, \
             tc.tile_pool(name="workE", bufs=14) as workE, \
             tc.tile_pool(name="workZ", bufs=8) as workZ, \
             tc.tile_pool(name="work2", bufs=4) as work2, \
             tc.tile_pool(name="workO", bufs=4) as workO, \
             tc.tile_pool(name="dramP", bufs=3, space="DRAM") as dramP, \
             tc.tile_pool(name="psP", bufs=2, space="PSUM") as psP, \
             tc.tile_pool(name="psS", bufs=1, space="PSUM") as psS, \
             tc.tile_pool(name="psZ", bufs=1, space="PSUM") as psZ:

            QT = [persist.tile([128, S], BF16, tag=f"qt{m}", name=f"qt{m}") for m in range(3)]
            KT = [persist.tile([128, S], BF16, tag=f"kt{m}", name=f"kt{m}") for m in range(3)]
            ZT = [persist.tile([128, S], BF16, tag=f"zt{m}", name=f"zt{m}") for m in range(3)]
            VA = [persist.tile([128, NHC * 65], BF16, tag=f"va{s}", name=f"va{s}") for s in range(16)]
            WO = [persist.tile([128, D], BF16, tag=f"wo{m}", name=f"wo{m}") for m in range(3)]
            MSK = persist.tile([128, 128], BF16, tag="mask", name="mask_sb")
            XT = [persist.tile([128, S], BF16, tag=f"xt{k}", name=f"xt{k}") for k in range(KD)]
            WQs = [persist.tile([128, HE], BF16, tag=f"wq{k}", name=f"wq{k}") for k in range(KD)]
            WKs = [persist.tile([128, HE], BF16, tag=f"wk{k}", name=f"wk{k}") for k in range(KD)]
            WVs = [persist.tile([128, HE], BF16, tag=f"wv{k}", name=f"wv{k}") for k in range(KD)]

            # issue order matters: the Sync engine issues dma_starts serially
            # (~340ns each), so the first Q/K projection group's inputs
            # (XT[k], WQs[k], WKs[k]) go first, in consumption order.
            for k in range(KD):
                nc.sync.dma_start(out=XT[k][:], in_=xt[k * 128:(k + 1) * 128, :])
                nc.sync.dma_start(out=WQs[k][:], in_=wq[k * 128:(k + 1) * 128, :])
                nc.sync.dma_start(out=WKs[k][:], in_=wk[k * 128:(k + 1) * 128, :])
            for k in range(KD):
                nc.sync.dma_start(out=WVs[k][:], in_=wv[k * 128:(k + 1) * 128, :])
            for m in range(3):
                nc.sync.dma_start(out=WO[m][:], in_=wo[m * 128:(m + 1) * 128, :])
            nc.sync.dma_start(out=MSK[:], in_=mask[:])
            ONES = persist.tile([1, 128], BF16, tag="ones", name="ones_sb")
            nc.vector.memset(ONES[:], 1.0)
            DUM = persist.tile([128, 512], BF16, tag="dum", name="dum_sb")
            nc.vector.memset(DUM[:], 1.0)

            def warm(n):
                # dependency-free matmuls: keep the PE busy (and the HAM
                # clock un-throttled) across stalls the scheduler can't fill
                # -- startup DMA waits and the tail normalize chain.  K must
                # be 128 (full rows): a thin-K stream reads as LOW activity
                # and actively cools the clock.
                for _ in range(n):
                    d = psP.tile([128, 512], F32, tag="psP", name="ps_warm")
                    nc.tensor.matmul(d[:], DUM[:, 0:128], DUM[:],
                                     start=True, stop=True)
            if qkv_bias:
                BQ = persist.tile([128, 3], F32, tag="bq", name="bq_sb")
                BK = persist.tile([128, 3], F32, tag="bk", name="bk_sb")
                BV = persist.tile([1, HE], BF16, tag="bv", name="bv_sb")
                for m in range(3):
                    nc.sync.dma_start(out=BQ[:, m:m + 1], in_=bq[m * 128:(m + 1) * 128, :])
                    nc.sync.dma_start(out=BK[:, m:m + 1], in_=bk[m * 128:(m + 1) * 128, :])
                nc.sync.dma_start(out=BV[:], in_=bv[:])
            for sc in range(16):
                nc.vector.memset(VA[sc][:], 1.0)

            # ---------------- interleave items ----------------
            # Filler work is split to single-matmul granularity (~200ns per
            # item) so feed() can smooth the PE load between attention blocks
            # -- chunky fillers leave micro-gaps that re-throttle the HAM
            # clock.  Items of one accumulation group share the psum tile via
            # a closure dict and stay adjacent in GQ order.
            def qk_items(is_q, m, n):
                st = {}
                items = []

                def mk(k):
                    def go():
                        if k == 0:
                            st["ps"] = psP.tile([128, 512], F32, tag="psP",
                                                name="ps_p")
                        Wt = WQs if is_q else WKs
                        nc.tensor.matmul(
                            st["ps"][:],
                            Wt[k][:, m * 128:(m + 1) * 128],
                            XT[k][:, n * 512:(n + 1) * 512],
                            start=(k == 0), stop=(k == KD - 1))
                    return go
                for k in range(KD):
                    items.append(mk(k))

                def cp():
                    dst = (QT if is_q else KT)[m][:, n * 512:(n + 1) * 512]
                    if qkv_bias:
                        bias_t = BQ if is_q else BK
                        nc.scalar.activation(dst, st["ps"][:], AF.Copy,
                                             bias=bias_t[:, m:m + 1])
                    else:
                        nc.vector.tensor_copy(dst, st["ps"][:])
                items.append(cp)
                return items

            def v_items(sc):
                st = {}
                items = []
                last_mm = KD - 1

                def mk(k):
                    def go():
                        if k == 0:
                            st["ps"] = psP.tile([128, 512], F32, tag="psP",
                                                name="ps_p")
                        nc.tensor.matmul(
                            st["ps"][:, 0:HE],
                            XT[k][:, sc * 128:(sc + 1) * 128],
                            WVs[k][:],
                            start=(k == 0),
                            stop=False if qkv_bias else (k == last_mm))
                    return go
                for k in range(KD):
                    items.append(mk(k))
                if qkv_bias:
                    items.append(lambda: nc.tensor.matmul(
                        st["ps"][:, 0:HE], ONES[:], BV[:],
                        start=False, stop=True))

                def cp():
                    nc.vector.tensor_copy(
                        VA[sc][:].rearrange("p (h c) -> p h c", c=65)[:, :, 0:64],
                        st["ps"][:, 0:HE].rearrange("p (h c) -> p h c", c=64))
                items.append(cp)
                return items

            def out_items(mc, half):
                st = {}
                items = []
                n0 = half * HE

                def mk(k):
                    def go():
                        if k == 0:
                            st["ps"] = psP.tile([128, 512], F32, tag="psP",
                                                name="ps_p")
                        nc.tensor.matmul(
                            st["ps"][:, 0:HE],
                            ZT[k][:, mc * 128:(mc + 1) * 128],
                            WO[k][:, n0:n0 + HE],
                            start=(k == 0), stop=(k == 2))
                    return go
                for k in range(3):
                    items.append(mk(k))

                def cp():
                    ob = workO.tile([128, HE], F32, tag="ob", name="ob")
                    nc.vector.tensor_copy(ob[:], st["ps"][:, 0:HE])
                    nc.sync.dma_start(
                        out=out[mc * 128:(mc + 1) * 128, n0:n0 + HE],
                        in_=ob[:])
                items.append(cp)
                return items

            def out_split_items(mc, half):
                st = {}
                n0 = half * HE

                def p0():
                    st["ps"] = psP.tile([128, 512], F32, tag="psP",
                                        name="ps_p")
                    nc.tensor.matmul(
                        st["ps"][:, 0:HE],
                        ZT[0][:, mc * 128:(mc + 1) * 128],
                        WO[0][:, n0:n0 + HE], start=True, stop=False)

                def p1():
                    nc.tensor.matmul(
                        st["ps"][:, 0:HE],
                        ZT[1][:, mc * 128:(mc + 1) * 128],
                        WO[1][:, n0:n0 + HE], start=False, stop=True)

                def pcp():
                    st["op"] = workO.tile([128, HE], F32, tag="op01",
                                          name="op01", bufs=8)
                    nc.vector.tensor_copy(st["op"][:], st["ps"][:, 0:HE])

                def f0():
                    st["ps2"] = psP.tile([128, 512], F32, tag="psP",
                                         name="ps_p")
                    nc.tensor.matmul(
                        st["ps2"][:, 0:HE],
                        ZT[2][:, mc * 128:(mc + 1) * 128],
                        WO[2][:, n0:n0 + HE], start=True, stop=True)

                def f1():
                    ob = workO.tile([128, HE], F32, tag="ob", name="ob")
                    nc.vector.tensor_add(ob[:], st["ps2"][:, 0:HE],
                                         st["op"][:])
                    nc.sync.dma_start(
                        out=out[mc * 128:(mc + 1) * 128, n0:n0 + HE],
                        in_=ob[:])
                return [p0, p1, pcp], [f0, f1]

            # Global interleave queue: (deadline, pred, fn).
            #  - deadline (hp, t, 0): the item WRITES a tile some score block
            #    of sweep (hp, t) reads, so it must be emitted before that
            #    sweep starts (emission-order WAR hazard otherwise).
            #    Enforced by drain() at every sweep start.
            #  - pred(pos) -> bool: earliest feed() may emit it.  Position
            #    gates reserve fillers for later, PE-thin phases; out-proj
            #    items are gated on the ZT writes they read being emitted
            #    (zt_ready counts normalize part2s per (hp, t)).
            # Q/K chunk n feeds sweep t=n of its pair; V[sc] feeds pair-0
            # sweep sc//4.
            GQ = []
            FAR = (9, 9, 9)
            zt_ready = {}

            def after(mark):
                return lambda pos: pos >= mark

            def zt_done(tq):
                return lambda pos: zt_ready.get((2, tq), 0) >= 2

            def put(deadline, pred, items):
                # one GQ entry per GROUP: the group's matmuls stay adjacent
                # in the PE stream (splitting them destroys LDWEIGHTS
                # pipelining); credit is spent per-item so pacing stays fine.
                GQ.append((deadline, pred, items, len(items)))

            always = after((-1, -1, -1))

            def sweep_before(m, n):
                # deadline one sweep EARLY: proj tiles must be written well
                # before the first score block that reads them, or the score
                # stalls on the projection's fresh psum->SBUF copy.
                return (m, n - 1, 0) if n >= 1 else (m - 1, 3, 0)

            # pair-0 remainder, ordered V-first within each deadline class
            for n in (2, 3):
                for sc in range(4 * n, 4 * n + 4):
                    put(sweep_before(0, n), always, v_items(sc))
                put(sweep_before(0, n), always, qk_items(True, 0, n))
                put(sweep_before(0, n), always, qk_items(False, 0, n))
            for n in range(4):
                put(sweep_before(1, n), always, qk_items(True, 1, n))
                put(sweep_before(1, n), always, qk_items(False, 1, n))
            # pair-2 projections: n0/n1 reserved for D(1), n2/n3 for D(2)'s
            # first two sweeps (which otherwise have no filler work).
            for n in range(4):
                pred = after((1, 0, 0)) if n < 2 else after((2, 0, 0))
                put(sweep_before(2, n), pred, qk_items(True, 2, n))
                put(sweep_before(2, n), pred, qk_items(False, 2, n))
            for tq in range(3):
                for mc in range(4 * tq, 4 * tq + 4):
                    for half in (0, 1):
                        put(FAR, zt_done(tq), out_items(mc, half))
            # tq=3 split: the ZT[0]/ZT[1] contributions run during sweep 3
            # (partial into SBUF); only the ZT[2] matmul + add + DMA wait for
            # the last normalize, shrinking the tail stall.
            _tq3 = [(mc, half) for mc in range(12, 16) for half in (0, 1)]
            _splits = {k: out_split_items(*k) for k in _tq3}
            for k in _tq3:
                put(FAR, after((2, 3, 0)), _splits[k][0])
            for k in _tq3:
                put(FAR, zt_done(3), _splits[k][1])

            feed_credit = [0.0]

            def run_group(entry):
                for it in entry[2]:
                    it()

            def feed(pos, r):
                feed_credit[0] += r
                while GQ and feed_credit[0] >= GQ[0][3] and GQ[0][1](pos):
                    e = GQ.pop(0)
                    run_group(e)
                    feed_credit[0] -= e[3]

            def drain(pos):
                # deadline-forced emission (sweep-start prerequisites)
                while GQ and GQ[0][0] <= pos:
                    run_group(GQ.pop(0))

            def drain_all():
                while GQ:
                    run_group(GQ.pop(0))

            # ---------------- attention stream ----------------
            # av_q is GLOBAL: the deferred-AV lag flows across pair
            # boundaries, so a new pair's (independent) score blocks and
            # sweep-start filler bursts run while the old pair's last exps
            # finish, instead of the PE head-of-line blocking on them.
            av_q = []      # aged deferred work: (fn, norm_fn | None)

            def pump_avs(lag):
                while len(av_q) > lag:
                    av_fn, norm_fn = av_q.pop(0)
                    av_fn()
                    if norm_fn is not None:
                        norm_fn()

            def run_pair(hp, rate):
                psz = {}

                def emit_normalize(par, t):
                    ho = par * 64
                    # drain psz out of PSUM fast (frees the bank)
                    zraw = workZ.tile([65, 512], BF16, tag="zraw", name="zraw")
                    nc.vector.tensor_copy(zraw[:], psz[par][:])

                    # reciprocal of the denominator row, reshaped across all
                    # 128 DVE lanes via a direct SBUF->SBUF strided DMA
                    # ([1,512] row -> [128,4]); a 1-lane [1,512] reciprocal
                    # costs 3.3us.
                    zr = zraw[64:65, :]
                    rp = work2.tile([128, 4], BF16, tag="rp", name="rp")
                    nc.sync.dma_start(out=rp[:], in_=bass.AP(
                        tensor=zr.tensor, offset=zr.offset,
                        ap=[zr.ap[0], [4, 128], [1, 4]]))
                    rcp = work2.tile([128, 4], BF16, tag="rcp", name="rcp")
                    with nc.allow_low_precision(reason="softmax recip bf16"):
                        nc.vector.reciprocal(rcp[:], rp[:])
                    rcd = dramP.tile([1, 512], BF16, tag="rcd", name="rcd")
                    rcd_ap = rcd[:]
                    nc.sync.dma_start(out=bass.AP(
                        tensor=rcd_ap.tensor, offset=rcd_ap.offset,
                        ap=[[4, 128], [1, 4]]), in_=rcp[:])
                    bc = work2.tile([64, 512], BF16, tag="bc", name="bc")
                    nc.sync.dma_start(out=bc[:], in_=bass.AP(
                        tensor=rcd_ap.tensor, offset=rcd_ap.offset,
                        ap=[[0, 64], rcd_ap.ap[-1]]))

                    def part2():
                        # final scale, re-queued behind AV_LAG more items so
                        # the reciprocal's DMA-bounce latency is hidden
                        nc.vector.tensor_mul(
                            ZT[hp][ho:ho + 64, t * 512:(t + 1) * 512],
                            zraw[0:64, :], bc[:])
                        zt_ready[(hp, t)] = zt_ready.get((hp, t), 0) + 1
                    av_q.append((part2, None))

                for t in range(4):
                    drain((hp, t, 0))
                    for j in range(4 * t + 4):
                        r = j - 4 * t  # >= 0 only on the diagonal
                        q0 = 128 * r if r >= 0 else 0
                        # Both heads' scores in ONE psum tile, anchored at the
                        # bank boundary: par0 writes [q0, 512) (bank 0), par1
                        # [512, 512+w) (bank 1).  The two matmuls run
                        # concurrently (disjoint row groups) so they must hit
                        # DIFFERENT single-port psum banks, and the regions
                        # stay contiguous so one exp instruction covers both.
                        w = 512 - q0
                        c0s = (q0, 512)
                        pss = psS.tile([128, 1024], F32, tag="pss", name="pss")
                        for par in (0, 1):
                            ho = par * 64
                            nc.tensor.matmul(
                                pss[:, c0s[par]:c0s[par] + w],
                                KT[hp][ho:ho + 64, j * 128:(j + 1) * 128],
                                QT[hp][ho:ho + 64, t * 512 + q0:(t + 1) * 512],
                                start=True, stop=True)
                        et = workE.tile([128, 1024], BF16, tag="et", name="et")
                        nc.scalar.activation(et[:, q0:512 + w],
                                             pss[:, q0:512 + w],
                                             AF.Exp, scale=0.125)
                        if r >= 0:
                            # gpsimd (otherwise idle): keeps the DVE queue
                            # short so AV-gating work isn't behind proj copies
                            for c0 in c0s:
                                nc.gpsimd.tensor_mul(
                                    et[:, c0:c0 + 128],
                                    et[:, c0:c0 + 128],
                                    MSK[:])
                        for par in (0, 1):
                            a0 = c0s[par]  # valid region start in et

                            def av_fn(par=par, et=et, j=j, t=t, a0=a0, q0=q0):
                                if j == 0:
                                    psz[par] = psZ.tile(
                                        [65, 512], F32,
                                        tag=f"psz{par}{t % 2}",
                                        name=f"psz{par}{t % 2}")
                                h = 2 * hp + par
                                nc.tensor.matmul(
                                    psz[par][:, q0:512],
                                    VA[j][:, h * 65:(h + 1) * 65],
                                    et[:, a0:a0 + 512 - q0],
                                    start=(j == 0), stop=(j == 4 * t + 3))
                            norm_fn = (
                                lambda par=par, t=t:
                                emit_normalize(par, t)) if j == 4 * t + 3 else None
                            av_q.append((av_fn, norm_fn))
                        feed((hp, t, j), rate)
                        pump_avs(AV_LAG)
                    pump_avs(2)

            # upfront projections for pair 0's first two sweeps (PE-only
            # prologue; everything else interleaves into the D streams).
            for n in (0, 1):
                for it in qk_items(True, 0, n) + qk_items(False, 0, n):
                    it()
            for sc in range(8):
                for it in v_items(sc):
                    it()

            run_pair(0, 2.6)
            run_pair(1, 2.2)
            run_pair(2, 3.2)
            pump_avs(0)
            drain_all()
    nc.compile()
    return nc


def _build_mask() -> np.ndarray:
    # triangle for the strict-diagonal 128x128 strip: 1.0 iff q_local >= k_local
    kl = np.arange(128)[:, None]
    ql = np.arange(128)[None, :]
    return (ql >= kl).astype(np.float32)


def kernel(**inputs) -> np.ndarray:
    global LAST_EXEC_TIME_NS
    x = np.asarray(inputs["normalized_resid_pre"], dtype=np.float32)
    W_Q = np.asarray(inputs["W_Q"], dtype=np.float32)
    W_K = np.asarray(inputs["W_K"], dtype=np.float32)
    W_V = np.asarray(inputs["W_V"], dtype=np.float32)
    W_O = np.asarray(inputs["W_O"], dtype=np.float32)
    b_Q = np.asarray(inputs["b_Q"], dtype=np.float32)
    b_K = np.asarray(inputs["b_K"], dtype=np.float32)
    b_V = np.asarray(inputs["b_V"], dtype=np.float32)
    b_O = np.asarray(inputs["b_O"], dtype=np.float32)

    qkv_bias = bool(b_Q.any() or b_K.any() or b_V.any())
    key = qkv_bias
    if key not in _GRAPH_CACHE:
        _GRAPH_CACHE[key] = _build_graph(qkv_bias)
    nc = _GRAPH_CACHE[key]

    mask = _build_mask()
    in_maps = []
    for c in range(8):
        b, h0 = c // 2, NHC * (c % 2)
        im = {
            "xt": np.ascontiguousarray(x[b].T).astype(BF16NP),
            "wq": np.ascontiguousarray(
                W_Q[h0:h0 + NHC].transpose(1, 0, 2).reshape(D, HE)).astype(BF16NP),
            "wk": np.ascontiguousarray(
                W_K[h0:h0 + NHC].transpose(1, 0, 2).reshape(D, HE)).astype(BF16NP),
            "wv": np.ascontiguousarray(
                W_V[h0:h0 + NHC].transpose(1, 0, 2).reshape(D, HE)).astype(BF16NP),
            "wo": np.ascontiguousarray(W_O[h0:h0 + NHC].reshape(HE, D)).astype(BF16NP),
            "mask": mask.astype(BF16NP),
        }
        if qkv_bias:
            im["bq"] = np.ascontiguousarray(b_Q[h0:h0 + NHC].reshape(HE, 1))
            im["bk"] = np.ascontiguousarray(b_K[h0:h0 + NHC].reshape(HE, 1))
            im["bv"] = np.ascontiguousarray(b_V[h0:h0 + NHC].reshape(1, HE)).astype(BF16NP)
        in_maps.append(im)

    import os
    trace = bool(os.environ.get("KERNEL_TRACE"))
    res = run_bass_kernel_spmd(nc, in_maps, core_ids=list(range(8)), trace=trace)
    LAST_EXEC_TIME_NS = res.exec_time_ns
    results = res.results

    out = np.empty((B, S, D), dtype=np.float32)
    for b in range(B):
        out[b] = results[2 * b]["out"] + results[2 * b + 1]["out"]
    if b_O.any():
        out += b_O
    return out


# revision 38
# speedup vs baseline: 1.2093x; 1.2093x over previous
"""Causal multi-head attention on 8 TRN2 NeuronCores.

Problem: x[4, 2048, 768], 12 heads x d_head 64, causal softmax attention.

Sharding: core c handles batch b = c//2 and the 6-head group h0 = 6*(c%2).
Each core computes its partial output o_partial[2048, 768] = sum over its 6
heads of (softmax(QK^T/8) V) @ W_O.  The two cores sharing a batch are summed
on the host (part of unsharding), so the device graph needs no collectives.

Device layout (per core) avoids every on-chip transpose:
  - host passes x^T  [768, 2048]  (xt)
  - Q^T, K^T [384, 2048] = W^T @ x^T   (lhsT = W slice, rhs = x^T)
  - V        [2048, 384]  = x @ W_V    (lhsT = x^T slice, rhs = W_V)
  - scores^T blocks [128k, 512q] = (K^T slice).T @ Q^T slice  (per head, K=64)
  - exp on ACT (scale=1/8 folded in); causal mask = 0/1 multiply on diagonal
    blocks; V is stored augmented with a ones column per head so the single
    AV matmul produces both z^T rows (64) and the softmax denominators (row 64)
  - z^T normalized via DMA-broadcast reciprocal row, stored as ZT [384, 2048]
  - out = (ZT).T @ W_O   (lhsT = ZT slice, rhs = W_O)
All matmuls run in bf16 (1 cycle/column vs 4 for fp32; f32 PSUM accum).

Scheduling: the attention phase is ACT(exp)-bound (~35us of exp per head
pair vs ~25us of PE), while projections and the output projection are
PE-only.  So emission interleaves them: while pair hp's attention stream
keeps ACT busy, the PE also runs pair hp+1's Q/K projections (and, during
the last pair, the output-projection tiles), paced by a credit-fed global
queue with per-sweep deadlines (emission-order WAR safety) and eligibility
gates (fillers reserved for PE-thin phases; out-proj gated on the ZT writes
it reads being emitted).  q-supertiles are processed one at a time (not in
halves) so PSUM fits scores (4 banks) + z accumulators (2) + a
projection/output slot (2).  Every score block writes the two heads'
columns anchored at the psum bank boundary (par0 [q0,512) in bank 0, par1
[512,512+w) in bank 1): the concurrent matmuls hit different single-port
banks, and one exp instruction covers both heads.  Deferred AV / normalize
closures ride a global lag queue that flows across sweep and pair
boundaries; causal-mask multiplies run on the otherwise-idle GpSimd engine;
the last supertile's output projection is split so only the ZT[2] matmul
waits on the final normalize.

Note: the chip drops the PE from 2.4 to ~2.0 GHz (P0 power state) under
sustained load, so back-to-back benchmark runs measure ~10-13% slower than
cooled runs (~236us cooled, ~268us hot).
"""

import sys

if "/opt/trn_rl_repo" not in sys.path:
    sys.path.insert(0, "/opt/trn_rl_repo")

import numpy as np
import ml_dtypes

BF16NP = ml_dtypes.bfloat16


def _ensure_ntff_hook():
    """The agent image's `antenv` lacks `axon_hooks`, which bass_utils needs
    for trace=True under axon. Recreate it via sys.modules injection using the
    boot helper's ctypes wrapper around libaxon_pjrt.so."""
    import types
    if "antenv.axon_hooks" in sys.modules:
        return
    try:
        from trn_agent_boot.trn_boot import _ntff_profile_via_ctypes
        hook = _ntff_profile_via_ctypes("/opt/axon/libaxon_pjrt.so")
    except Exception:
        hook = None
    m = types.ModuleType("antenv.axon_hooks")
    m._hook = hook
    m.get_axon_ntff_profile_hook = lambda: m._hook
    def _set(h):
        m._hook = h
    m.set_axon_ntff_profile_hook = _set
    sys.modules["antenv.axon_hooks"] = m


_ensure_ntff_hook()

import concourse.bass as bass
import concourse.tile as tile
from concourse import bacc, mybir
from concourse.bass_utils import run_bass_kernel_spmd

F32 = mybir.dt.float32
BF16 = mybir.dt.bfloat16
AF = mybir.ActivationFunctionType

D = 768          # d_model
S = 2048         # seq
E = 64           # d_head
NHC = 6          # heads per core
HE = NHC * E     # 384
KD = D // 128    # 6 k-chunks over d_model
B = 4

AV_LAG = 8       # deferred-AV depth (keeps PE from stalling on exp)

LAST_EXEC_TIME_NS = None
_GRAPH_CACHE = {}


def _build_graph(qkv_bias: bool) -> bass.Bass:
    nc = bacc.Bacc("TRN2", target_bir_lowering=False)
    xt = nc.declare_dram_parameter("xt", [D, S], BF16, isOutput=False)
    wq = nc.declare_dram_parameter("wq", [D, HE], BF16, isOutput=False)
    wk = nc.declare_dram_parameter("wk", [D, HE], BF16, isOutput=False)
    wv = nc.declare_dram_parameter("wv", [D, HE], BF16, isOutput=False)
    wo = nc.declare_dram_parameter("wo", [HE, D], BF16, isOutput=False)
    mask = nc.declare_dram_parameter("mask", [128, 128], BF16, isOutput=False)
    if qkv_bias:
        bq = nc.declare_dram_parameter("bq", [HE, 1], F32, isOutput=False)
        bk = nc.declare_dram_parameter("bk", [HE, 1], F32, isOutput=False)
        bv = nc.declare_dram_parameter("bv", [1, HE], BF16, isOutput=False)
    out = nc.declare_dram_parameter("out", [S, D], F32, isOutput=True)

    with tile.TileContext(nc) as tc:
        with tc.tile_pool(name="persist", bufs=1) as persist, \
             tc.tile_pool(name="workE", bufs=14) as workE, \
             tc.tile_pool(name="workZ", bufs=8) as workZ, \
             tc.tile_pool(name="work2", bufs=4) as work2, \
             tc.tile_pool(name="workO", bufs=4) as workO, \
             tc.tile_pool(name="dramP", bufs=3, space="DRAM") as dramP, \
             tc.tile_pool(name="psP", bufs=2, space="PSUM") as psP, \
             tc.tile_pool(name="psS", bufs=2, space="PSUM") as psS, \
             tc.tile_pool(name="psZ", bufs=1, space="PSUM") as psZ:

            QT = [persist.tile([128, S], BF16, tag=f"qt{m}", name=f"qt{m}") for m in range(3)]
            KT = [persist.tile([128, S], BF16, tag=f"kt{m}", name=f"kt{m}") for m in range(3)]
            ZT = [persist.tile([128, S], BF16, tag=f"zt{m}", name=f"zt{m}") for m in range(3)]
            VA = [persist.tile([128, NHC * 65], BF16, tag=f"va{s}", name=f"va{s}") for s in range(16)]
            WO = [persist.tile([128, D], BF16, tag=f"wo{m}", name=f"wo{m}") for m in range(3)]
            MSK = persist.tile([128, 128], BF16, tag="mask", name="mask_sb")
            XT = [persist.tile([128, S], BF16, tag=f"xt{k}", name=f"xt{k}") for k in range(KD)]
            WQs = [persist.tile([128, HE], BF16, tag=f"wq{k}", name=f"wq{k}") for k in range(KD)]
            WKs = [persist.tile([128, HE], BF16, tag=f"wk{k}", name=f"wk{k}") for k in range(KD)]
            WVs = [persist.tile([128, HE], BF16, tag=f"wv{k}", name=f"wv{k}") for k in range(KD)]

            # issue order matters: the Sync engine issues dma_starts serially
            # (~340ns each), so the first Q/K projection group's inputs
            # (XT[k], WQs[k], WKs[k]) go first, in consumption order.
            for k in range(KD):
                nc.sync.dma_start(out=XT[k][:], in_=xt[k * 128:(k + 1) * 128, :])
                nc.sync.dma_start(out=WQs[k][:], in_=wq[k * 128:(k + 1) * 128, :])
                nc.sync.dma_start(out=WKs[k][:], in_=wk[k * 128:(k + 1) * 128, :])
            for k in range(KD):
                nc.sync.dma_start(out=WVs[k][:], in_=wv[k * 128:(k + 1) * 128, :])
            for m in range(3):
                nc.sync.dma_start(out=WO[m][:], in_=wo[m * 128:(m + 1) * 128, :])
            nc.sync.dma_start(out=MSK[:], in_=mask[:])
            ONES = persist.tile([1, 128], BF16, tag="ones", name="ones_sb")
            nc.vector.memset(ONES[:], 1.0)
            DUM = persist.tile([128, 512], BF16, tag="dum", name="dum_sb")
            nc.vector.memset(DUM[:], 1.0)

            def warm(n):
                # dependency-free matmuls: keep the PE busy (and the HAM
                # clock un-throttled) across stalls the scheduler can't fill
                # -- startup DMA waits and the tail normalize chain.  K must
                # be 128 (full rows): a thin-K stream reads as LOW activity
                # and actively cools the clock.
                for _ in range(n):
                    d = psP.tile([128, 512], F32, tag="psP", name="ps_warm")
                    nc.tensor.matmul(d[:], DUM[:, 0:128], DUM[:],
                                     start=True, stop=True)
            if qkv_bias:
                BQ = persist.tile([128, 3], F32, tag="bq", name="bq_sb")
                BK = persist.tile([128, 3], F32, tag="bk", name="bk_sb")
                BV = persist.tile([1, HE], BF16, tag="bv", name="bv_sb")
                for m in range(3):
                    nc.sync.dma_start(out=BQ[:, m:m + 1], in_=bq[m * 128:(m + 1) * 128, :])
                    nc.sync.dma_start(out=BK[:, m:m + 1], in_=bk[m * 128:(m + 1) * 128, :])
                nc.sync.dma_start(out=BV[:], in_=bv[:])
            for sc in range(16):
                nc.vector.memset(VA[sc][:], 1.0)

            # ---------------- interleave items ----------------
            # Filler work is split to single-matmul granularity (~200ns per
            # item) so feed() can smooth the PE load between attention blocks
            # -- chunky fillers leave micro-gaps that re-throttle the HAM
            # clock.  Items of one accumulation group share the psum tile via
            # a closure dict and stay adjacent in GQ order.
            def qk_items(is_q, m, n):
                st = {}
                items = []

                def mk(k):
                    def go():
                        if k == 0:
                            st["ps"] = psP.tile([128, 512], F32, tag="psP",
                                                name="ps_p")
                        Wt = WQs if is_q else WKs
                        nc.tensor.matmul(
                            st["ps"][:],
                            Wt[k][:, m * 128:(m + 1) * 128],
                            XT[k][:, n * 512:(n + 1) * 512],
                            start=(k == 0), stop=(k == KD - 1))
                    return go
                for k in range(KD):
                    items.append(mk(k))

                def cp():
                    dst = (QT if is_q else KT)[m][:, n * 512:(n + 1) * 512]
                    if qkv_bias:
                        bias_t = BQ if is_q else BK
                        nc.scalar.activation(dst, st["ps"][:], AF.Copy,
                                             bias=bias_t[:, m:m + 1])
                    else:
                        nc.vector.tensor_copy(dst, st["ps"][:])
                items.append(cp)
                return items

            def v_items(sc):
                st = {}
                items = []
                last_mm = KD - 1

                def mk(k):
                    def go():
                        if k == 0:
                            st["ps"] = psP.tile([128, 512], F32, tag="psP",
                                                name="ps_p")
                        nc.tensor.matmul(
                            st["ps"][:, 0:HE],
                            XT[k][:, sc * 128:(sc + 1) * 128],
                            WVs[k][:],
                            start=(k == 0),
                            stop=False if qkv_bias else (k == last_mm))
                    return go
                for k in range(KD):
                    items.append(mk(k))
                if qkv_bias:
                    items.append(lambda: nc.tensor.matmul(
                        st["ps"][:, 0:HE], ONES[:], BV[:],
                        start=False, stop=True))

                def cp():
                    nc.vector.tensor_copy(
                        VA[sc][:].rearrange("p (h c) -> p h c", c=65)[:, :, 0:64],
                        st["ps"][:, 0:HE].rearrange("p (h c) -> p h c", c=64))
                items.append(cp)
                return items

            def out_items(mc, half):
                st = {}
                items = []
                n0 = half * HE

                def mk(k):
                    def go():
                        if k == 0:
                            st["ps"] = psP.tile([128, 512], F32, tag="psP",
                                                name="ps_p")
                        nc.tensor.matmul(
                            st["ps"][:, 0:HE],
                            ZT[k][:, mc * 128:(mc + 1) * 128],
                            WO[k][:, n0:n0 + HE],
                            start=(k == 0), stop=(k == 2))
                    return go
                for k in range(3):
                    items.append(mk(k))

                def cp():
                    ob = workO.tile([128, HE], F32, tag="ob", name="ob")
                    nc.vector.tensor_copy(ob[:], st["ps"][:, 0:HE])
                    nc.sync.dma_start(
                        out=out[mc * 128:(mc + 1) * 128, n0:n0 + HE],
                        in_=ob[:])
                items.append(cp)
                return items

            def out_split_items(mc, half):
                st = {}
                n0 = half * HE

                def p0():
                    st["ps"] = psP.tile([128, 512], F32, tag="psP",
                                        name="ps_p")
                    nc.tensor.matmul(
                        st["ps"][:, 0:HE],
                        ZT[0][:, mc * 128:(mc + 1) * 128],
                        WO[0][:, n0:n0 + HE], start=True, stop=False)

                def p1():
                    nc.tensor.matmul(
                        st["ps"][:, 0:HE],
                        ZT[1][:, mc * 128:(mc + 1) * 128],
                        WO[1][:, n0:n0 + HE], start=False, stop=True)

                def pcp():
                    st["op"] = workO.tile([128, HE], F32, tag="op01",
                                          name="op01", bufs=8)
                    nc.vector.tensor_copy(st["op"][:], st["ps"][:, 0:HE])

                def f0():
                    st["ps2"] = psP.tile([128, 512], F32, tag="psP",
                                         name="ps_p")
                    nc.tensor.matmul(
                        st["ps2"][:, 0:HE],
                        ZT[2][:, mc * 128:(mc + 1) * 128],
                        WO[2][:, n0:n0 + HE], start=True, stop=True)

                def f1():
                    ob = workO.tile([128, HE], F32, tag="ob", name="ob")
                    nc.vector.tensor_add(ob[:], st["ps2"][:, 0:HE],
                                         st["op"][:])
                    nc.sync.dma_start(
                        out=out[mc * 128:(mc + 1) * 128, n0:n0 + HE],
                        in_=ob[:])
                return [p0, p1, pcp], [f0, f1]

            # Global interleave queue: (deadline, pred, fn).
            #  - deadline (hp, t, 0): the item WRITES a tile some score block
            #    of sweep (hp, t) reads, so it must be emitted before that
            #    sweep starts (emission-order WAR hazard otherwise).
            #    Enforced by drain() at every sweep start.
            #  - pred(pos) -> bool: earliest feed() may emit it.  Position
            #    gates reserve fillers for later, PE-thin phases; out-proj
            #    items are gated on the ZT writes they read being emitted
            #    (zt_ready counts normalize part2s per (hp, t)).
            # Q/K chunk n feeds sweep t=n of its pair; V[sc] feeds pair-0
            # sweep sc//4.
            GQ = []
            FAR = (9, 9, 9)
            zt_ready = {}

            def after(mark):
                return lambda pos: pos >= mark

            def zt_done(tq):
                return lambda pos: zt_ready.get((2, tq), 0) >= 2

            def put(deadline, pred, items):
                # one GQ entry per GROUP: the group's matmuls stay adjacent
                # in the PE stream (splitting them destroys LDWEIGHTS
                # pipelining); credit is spent per-item so pacing stays fine.
                GQ.append((deadline, pred, items, len(items)))

            always = after((-1, -1, -1))

            def sweep_before(m, n):
                # deadline one sweep EARLY: proj tiles must be written well
                # before the first score block that reads them, or the score
                # stalls on the projection's fresh psum->SBUF copy.
                return (m, n - 1, 0) if n >= 1 else (m - 1, 3, 0)

            # pair-0 remainder, ordered V-first within each deadline class
            for n in (2, 3):
                for sc in range(4 * n, 4 * n + 4):
                    put(sweep_before(0, n), always, v_items(sc))
                put(sweep_before(0, n), always, qk_items(True, 0, n))
                put(sweep_before(0, n), always, qk_items(False, 0, n))
            for n in range(4):
                put(sweep_before(1, n), always, qk_items(True, 1, n))
                put(sweep_before(1, n), always, qk_items(False, 1, n))
            # pair-2 projections: n0/n1 reserved for D(1), n2/n3 for D(2)'s
            # first two sweeps (which otherwise have no filler work).
            for n in range(4):
                pred = after((1, 0, 0)) if n < 2 else after((2, 0, 0))
                put(sweep_before(2, n), pred, qk_items(True, 2, n))
                put(sweep_before(2, n), pred, qk_items(False, 2, n))
            for tq in range(3):
                for mc in range(4 * tq, 4 * tq + 4):
                    for half in (0, 1):
                        put(FAR, zt_done(tq), out_items(mc, half))
            # tq=3 split: the ZT[0]/ZT[1] contributions run during sweep 3
            # (partial into SBUF); only the ZT[2] matmul + add + DMA wait for
            # the last normalize, shrinking the tail stall.
            _tq3 = [(mc, half) for mc in range(12, 16) for half in (0, 1)]
            _splits = {k: out_split_items(*k) for k in _tq3}
            for k in _tq3:
                put(FAR, after((2, 3, 0)), _splits[k][0])
            for k in _tq3:
                put(FAR, zt_done(3), _splits[k][1])

            feed_credit = [0.0]

            def run_group(entry):
                for it in entry[2]:
                    it()

            def feed(pos, r):
                feed_credit[0] += r
                while GQ and feed_credit[0] >= GQ[0][3] and GQ[0][1](pos):
                    e = GQ.pop(0)
                    run_group(e)
                    feed_credit[0] -= e[3]

            def drain(pos):
                # deadline-forced emission (sweep-start prerequisites)
                while GQ and GQ[0][0] <= pos:
                    run_group(GQ.pop(0))

            def drain_all():
                while GQ:
                    run_group(GQ.pop(0))

            # ---------------- attention stream ----------------
            # av_q is GLOBAL: the deferred-AV lag flows across pair
            # boundaries, so a new pair's (independent) score blocks and
            # sweep-start filler bursts run while the old pair's last exps
            # finish, instead of the PE head-of-line blocking on them.
            av_q = []      # aged deferred work: (fn, norm_fn | None)

            def pump_avs(lag):
                while len(av_q) > lag:
                    av_fn, norm_fn = av_q.pop(0)
                    av_fn()
                    if norm_fn is not None:
                        norm_fn()

            def run_pair(hp, rate):
                psz = {}

                def emit_normalize(par, t):
                    ho = par * 64
                    # drain psz out of PSUM fast (frees the bank)
                    zraw = workZ.tile([65, 512], BF16, tag="zraw", name="zraw")
                    nc.vector.tensor_copy(zraw[:], psz[par][:])

                    # reciprocal of the denominator row, reshaped across all
                    # 128 DVE lanes via a direct SBUF->SBUF strided DMA
                    # ([1,512] row -> [128,4]); a 1-lane [1,512] reciprocal
                    # costs 3.3us.
                    zr = zraw[64:65, :]
                    rp = work2.tile([128, 4], BF16, tag="rp", name="rp")
                    nc.sync.dma_start(out=rp[:], in_=bass.AP(
                        tensor=zr.tensor, offset=zr.offset,
                        ap=[zr.ap[0], [4, 128], [1, 4]]))
                    rcp = work2.tile([128, 4], BF16, tag="rcp", name="rcp")
                    with nc.allow_low_precision(reason="softmax recip bf16"):
                        nc.vector.reciprocal(rcp[:], rp[:])
                    rcd = dramP.tile([1, 512], BF16, tag="rcd", name="rcd")
                    rcd_ap = rcd[:]
                    nc.sync.dma_start(out=bass.AP(
                        tensor=rcd_ap.tensor, offset=rcd_ap.offset,
                        ap=[[4, 128], [1, 4]]), in_=rcp[:])
                    bc = work2.tile([64, 512], BF16, tag="bc", name="bc")
                    nc.sync.dma_start(out=bc[:], in_=bass.AP(
                        tensor=rcd_ap.tensor, offset=rcd_ap.offset,
                        ap=[[0, 64], rcd_ap.ap[-1]]))

                    def part2():
                        # final scale, re-queued behind AV_LAG more items so
                        # the reciprocal's DMA-bounce latency is hidden
                        nc.vector.tensor_mul(
                            ZT[hp][ho:ho + 64, t * 512:(t + 1) * 512],
                            zraw[0:64, :], bc[:])
                        zt_ready[(hp, t)] = zt_ready.get((hp, t), 0) + 1
                    av_q.append((part2, None))

                for t in range(4):
                    drain((hp, t, 0))
                    for j in range(4 * t + 4):
                        r = j - 4 * t  # >= 0 only on the diagonal
                        q0 = 128 * r if r >= 0 else 0
                        # Both heads' scores in ONE psum tile, anchored at the
                        # bank boundary: par0 writes [q0, 512) (bank 0), par1
                        # [512, 512+w) (bank 1).  The two matmuls run
                        # concurrently (disjoint row groups) so they must hit
                        # DIFFERENT single-port psum banks, and the regions
                        # stay contiguous so one exp instruction covers both.
                        w = 512 - q0
                        c0s = (q0, 512)
                        pss = psS.tile([128, 1024], F32, tag="pss", name="pss")
                        for par in (0, 1):
                            ho = par * 64
                            nc.tensor.matmul(
                                pss[:, c0s[par]:c0s[par] + w],
                                KT[hp][ho:ho + 64, j * 128:(j + 1) * 128],
                                QT[hp][ho:ho + 64, t * 512 + q0:(t + 1) * 512],
                                start=True, stop=True)
                        et = workE.tile([128, 1024], BF16, tag="et", name="et")
                        nc.scalar.activation(et[:, q0:512 + w],
                                             pss[:, q0:512 + w],
                                             AF.Exp, scale=0.125)
                        if r >= 0:
                            # gpsimd (otherwise idle): keeps the DVE queue
                            # short so AV-gating work isn't behind proj copies
                            for c0 in c0s:
                                nc.gpsimd.tensor_mul(
                                    et[:, c0:c0 + 128],
                                    et[:, c0:c0 + 128],
                                    MSK[:])
                        for par in (0, 1):
                            a0 = c0s[par]  # valid region start in et

                            def av_fn(par=par, et=et, j=j, t=t, a0=a0, q0=q0):
                                if j == 0:
                                    psz[par] = psZ.tile(
                                        [65, 512], F32, tag=f"psz{par}",
                                        name=f"psz{par}")
                                h = 2 * hp + par
                                nc.tensor.matmul(
                                    psz[par][:, q0:512],
                                    VA[j][:, h * 65:(h + 1) * 65],
                                    et[:, a0:a0 + 512 - q0],
                                    start=(j == 0), stop=(j == 4 * t + 3))
                            norm_fn = (
                                lambda par=par, t=t:
                                emit_normalize(par, t)) if j == 4 * t + 3 else None
                            av_q.append((av_fn, norm_fn))
                        feed((hp, t, j), rate)
                        pump_avs(AV_LAG)
                    pump_avs(2)

            # upfront projections for pair 0's first two sweeps (PE-only
            # prologue; everything else interleaves into the D streams).
            for n in (0, 1):
                for it in qk_items(True, 0, n) + qk_items(False, 0, n):
                    it()
            for sc in range(8):
                for it in v_items(sc):
                    it()

            run_pair(0, 2.6)
            run_pair(1, 2.2)
            run_pair(2, 3.2)
            pump_avs(0)
            drain_all()
    nc.compile()
    return nc


def _build_mask() -> np.ndarray:
    # triangle for the strict-diagonal 128x128 strip: 1.0 iff q_local >= k_local
    kl = np.arange(128)[:, None]
    ql = np.arange(128)[None, :]
    return (ql >= kl).astype(np.float32)


def kernel(**inputs) -> np.ndarray:
    global LAST_EXEC_TIME_NS
    x = np.asarray(inputs["normalized_resid_pre"], dtype=np.float32)
    W_Q = np.asarray(inputs["W_Q"], dtype=np.float32)
    W_K = np.asarray(inputs["W_K"], dtype=np.float32)
    W_V = np.asarray(inputs["W_V"], dtype=np.float32)
    W_O = np.asarray(inputs["W_O"], dtype=np.float32)
    b_Q = np.asarray(inputs["b_Q"], dtype=np.float32)
    b_K = np.asarray(inputs["b_K"], dtype=np.float32)
    b_V = np.asarray(inputs["b_V"], dtype=np.float32)
    b_O = np.asarray(inputs["b_O"], dtype=np.float32)

    qkv_bias = bool(b_Q.any() or b_K.any() or b_V.any())
    key = qkv_bias
    if key not in _GRAPH_CACHE:
        _GRAPH_CACHE[key] = _build_graph(qkv_bias)
    nc = _GRAPH_CACHE[key]

    mask = _build_mask()
    in_maps = []
    for c in range(8):
        b, h0 = c // 2, NHC * (c % 2)
        im = {
            "xt": np.ascontiguousarray(x[b].T).astype(BF16NP),
            "wq": np.ascontiguousarray(
                W_Q[h0:h0 + NHC].transpose(1, 0, 2).reshape(D, HE)).astype(BF16NP),
            "wk": np.ascontiguousarray(
                W_K[h0:h0 + NHC].transpose(1, 0, 2).reshape(D, HE)).astype(BF16NP),
            "wv": np.ascontiguousarray(
                W_V[h0:h0 + NHC].transpose(1, 0, 2).reshape(D, HE)).astype(BF16NP),
            "wo": np.ascontiguousarray(W_O[h0:h0 + NHC].reshape(HE, D)).astype(BF16NP),
            "mask": mask.astype(BF16NP),
        }
        if qkv_bias:
            im["bq"] = np.ascontiguousarray(b_Q[h0:h0 + NHC].reshape(HE, 1))
            im["bk"] = np.ascontiguousarray(b_K[h0:h0 + NHC].reshape(HE, 1))
            im["bv"] = np.ascontiguousarray(b_V[h0:h0 + NHC].reshape(1, HE)).astype(BF16NP)
        in_maps.append(im)

    import os
    trace = bool(os.environ.get("KERNEL_TRACE"))
    res = run_bass_kernel_spmd(nc, in_maps, core_ids=list(range(8)), trace=trace)
    LAST_EXEC_TIME_NS = res.exec_time_ns
    results = res.results

    out = np.empty((B, S, D), dtype=np.float32)
    for b in range(B):
        out[b] = results[2 * b]["out"] + results[2 * b + 1]["out"]
    if b_O.any():
        out += b_O
    return out


# revision 42
# speedup vs baseline: 1.2498x; 1.0335x over previous
"""Causal multi-head attention on 8 TRN2 NeuronCores.

Problem: x[4, 2048, 768], 12 heads x d_head 64, causal softmax attention.

Sharding: core c handles batch b = c//2 and the 6-head group h0 = 6*(c%2).
Each core computes its partial output o_partial[2048, 768] = sum over its 6
heads of (softmax(QK^T/8) V) @ W_O.  The two cores sharing a batch are summed
on the host (part of unsharding), so the device graph needs no collectives.

Device layout (per core) avoids every on-chip transpose:
  - host passes x^T  [768, 2048]  (xt)
  - Q^T, K^T [384, 2048] = W^T @ x^T   (lhsT = W slice, rhs = x^T)
  - V        [2048, 384]  = x @ W_V    (lhsT = x^T slice, rhs = W_V)
  - scores^T blocks [128k, 512q] = (K^T slice).T @ Q^T slice  (per head, K=64)
  - exp on ACT (scale=1/8 folded in); causal mask = 0/1 multiply on diagonal
    blocks; V is stored augmented with a ones column per head so the single
    AV matmul produces both z^T rows (64) and the softmax denominators (row 64)
  - z^T normalized via DMA-broadcast reciprocal row, stored as ZT [384, 2048]
  - out = (ZT).T @ W_O   (lhsT = ZT slice, rhs = W_O)
All matmuls run in bf16 (1 cycle/column vs 4 for fp32; f32 PSUM accum).

Scheduling: the attention phase is ACT(exp)-bound (~35us of exp per head
pair vs ~25us of PE), while projections and the output projection are
PE-only.  So emission interleaves them: while pair hp's attention stream
keeps ACT busy, the PE also runs pair hp+1's Q/K projections (and, during
the last pair, the output-projection tiles), paced by a credit-fed global
queue with per-sweep deadlines (emission-order WAR safety) and eligibility
gates (fillers reserved for PE-thin phases; out-proj gated on the ZT writes
it reads being emitted).  q-supertiles are processed one at a time (not in
halves) so PSUM fits scores (4 banks) + z accumulators (2) + a
projection/output slot (2).  Every score block writes the two heads'
columns anchored at the psum bank boundary (par0 [q0,512) in bank 0, par1
[512,512+w) in bank 1): the concurrent matmuls hit different single-port
banks, and one exp instruction covers both heads.  Deferred AV / normalize
closures ride a global lag queue that flows across sweep and pair
boundaries; causal-mask multiplies run on the otherwise-idle GpSimd engine;
the last supertile's output projection is split so only the ZT[2] matmul
waits on the final normalize.

Note: the chip drops the PE from 2.4 to ~2.0 GHz (P0 power state) under
sustained load, so back-to-back benchmark runs measure ~10-13% slower than
cooled runs (~236us cooled, ~268us hot).
"""

import sys

if "/opt/trn_rl_repo" not in sys.path:
    sys.path.insert(0, "/opt/trn_rl_repo")

import numpy as np
import ml_dtypes

BF16NP = ml_dtypes.bfloat16


def _ensure_ntff_hook():
    """The agent image's `antenv` lacks `axon_hooks`, which bass_utils needs
    for trace=True under axon. Recreate it via sys.modules injection using the
    boot helper's ctypes wrapper around libaxon_pjrt.so."""
    import types
    if "antenv.axon_hooks" in sys.modules:
        return
    try:
        from trn_agent_boot.trn_boot import _ntff_profile_via_ctypes
        hook = _ntff_profile_via_ctypes("/opt/axon/libaxon_pjrt.so")
    except Exception:
        hook = None
    m = types.ModuleType("antenv.axon_hooks")
    m._hook = hook
    m.get_axon_ntff_profile_hook = lambda: m._hook
    def _set(h):
        m._hook = h
    m.set_axon_ntff_profile_hook = _set
    sys.modules["antenv.axon_hooks"] = m


_ensure_ntff_hook()

import concourse.bass as bass
import concourse.tile as tile
from concourse import bacc, mybir
from concourse.bass_utils import run_bass_kernel_spmd

F32 = mybir.dt.float32
BF16 = mybir.dt.bfloat16
AF = mybir.ActivationFunctionType

D = 768          # d_model
S = 2048         # seq
E = 64           # d_head
NHC = 6          # heads per core
HE = NHC * E     # 384
KD = D // 128    # 6 k-chunks over d_model
B = 4

AV_LAG = 8       # deferred-AV depth (keeps PE from stalling on exp)

LAST_EXEC_TIME_NS = None
_GRAPH_CACHE = {}


def _build_graph(qkv_bias: bool) -> bass.Bass:
    nc = bacc.Bacc("TRN2", target_bir_lowering=False)
    xt = nc.declare_dram_parameter("xt", [D, S], BF16, isOutput=False)
    wq = nc.declare_dram_parameter("wq", [D, HE], BF16, isOutput=False)
    wk = nc.declare_dram_parameter("wk", [D, HE], BF16, isOutput=False)
    wv = nc.declare_dram_parameter("wv", [D, HE], BF16, isOutput=False)
    wo = nc.declare_dram_parameter("wo", [HE, D], BF16, isOutput=False)
    mask = nc.declare_dram_parameter("mask", [128, 128], BF16, isOutput=False)
    if qkv_bias:
        bq = nc.declare_dram_parameter("bq", [HE, 1], F32, isOutput=False)
        bk = nc.declare_dram_parameter("bk", [HE, 1], F32, isOutput=False)
        bv = nc.declare_dram_parameter("bv", [1, HE], BF16, isOutput=False)
    out = nc.declare_dram_parameter("out", [S, D], F32, isOutput=True)

    with tile.TileContext(nc) as tc:
        with tc.tile_pool(name="persist", bufs=1) as persist, \
             tc.tile_pool(name="workE", bufs=14) as workE, \
             tc.tile_pool(name="workZ", bufs=8) as workZ, \
             tc.tile_pool(name="work2", bufs=4) as work2, \
             tc.tile_pool(name="workO", bufs=4) as workO, \
             tc.tile_pool(name="dramP", bufs=3, space="DRAM") as dramP, \
             tc.tile_pool(name="psP", bufs=2, space="PSUM") as psP, \
             tc.tile_pool(name="psS", bufs=2, space="PSUM") as psS, \
             tc.tile_pool(name="psZ", bufs=1, space="PSUM") as psZ:

            QT = [persist.tile([128, S], BF16, tag=f"qt{m}", name=f"qt{m}") for m in range(3)]
            KT = [persist.tile([128, S], BF16, tag=f"kt{m}", name=f"kt{m}") for m in range(3)]
            ZT = [persist.tile([128, S], BF16, tag=f"zt{m}", name=f"zt{m}") for m in range(3)]
            VA = [persist.tile([128, NHC * 65], BF16, tag=f"va{s}", name=f"va{s}") for s in range(16)]
            WO = [persist.tile([128, D], BF16, tag=f"wo{m}", name=f"wo{m}") for m in range(3)]
            MSK = persist.tile([128, 128], BF16, tag="mask", name="mask_sb")
            XT = [persist.tile([128, S], BF16, tag=f"xt{k}", name=f"xt{k}") for k in range(KD)]
            WQs = [persist.tile([128, HE], BF16, tag=f"wq{k}", name=f"wq{k}") for k in range(KD)]
            WKs = [persist.tile([128, HE], BF16, tag=f"wk{k}", name=f"wk{k}") for k in range(KD)]
            WVs = [persist.tile([128, HE], BF16, tag=f"wv{k}", name=f"wv{k}") for k in range(KD)]

            # issue order matters: the Sync engine issues dma_starts serially
            # (~340ns each), so the first Q/K projection group's inputs
            # (XT[k], WQs[k], WKs[k]) go first, in consumption order.
            for k in range(KD):
                nc.sync.dma_start(out=XT[k][:], in_=xt[k * 128:(k + 1) * 128, :])
                nc.sync.dma_start(out=WQs[k][:], in_=wq[k * 128:(k + 1) * 128, :])
                nc.sync.dma_start(out=WKs[k][:], in_=wk[k * 128:(k + 1) * 128, :])
            for k in range(KD):
                nc.sync.dma_start(out=WVs[k][:], in_=wv[k * 128:(k + 1) * 128, :])
            for m in range(3):
                nc.sync.dma_start(out=WO[m][:], in_=wo[m * 128:(m + 1) * 128, :])
            nc.sync.dma_start(out=MSK[:], in_=mask[:])
            ONES = persist.tile([1, 128], BF16, tag="ones", name="ones_sb")
            nc.vector.memset(ONES[:], 1.0)
            DUM = persist.tile([128, 512], BF16, tag="dum", name="dum_sb")
            nc.vector.memset(DUM[:], 1.0)

            def warm(n):
                # dependency-free matmuls: keep the PE busy (and the HAM
                # clock un-throttled) across stalls the scheduler can't fill
                # -- startup DMA waits and the tail normalize chain.  K must
                # be 128 (full rows): a thin-K stream reads as LOW activity
                # and actively cools the clock.
                for _ in range(n):
                    d = psP.tile([128, 512], F32, tag="psP", name="ps_warm")
                    nc.tensor.matmul(d[:], DUM[:, 0:128], DUM[:],
                                     start=True, stop=True)
            if qkv_bias:
                BQ = persist.tile([128, 3], F32, tag="bq", name="bq_sb")
                BK = persist.tile([128, 3], F32, tag="bk", name="bk_sb")
                BV = persist.tile([1, HE], BF16, tag="bv", name="bv_sb")
                for m in range(3):
                    nc.sync.dma_start(out=BQ[:, m:m + 1], in_=bq[m * 128:(m + 1) * 128, :])
                    nc.sync.dma_start(out=BK[:, m:m + 1], in_=bk[m * 128:(m + 1) * 128, :])
                nc.sync.dma_start(out=BV[:], in_=bv[:])
            for sc in range(16):
                nc.vector.memset(VA[sc][:], 1.0)

            # ---------------- interleave items ----------------
            # Filler work is split to single-matmul granularity (~200ns per
            # item) so feed() can smooth the PE load between attention blocks
            # -- chunky fillers leave micro-gaps that re-throttle the HAM
            # clock.  Items of one accumulation group share the psum tile via
            # a closure dict and stay adjacent in GQ order.
            def qk_items(is_q, m, n):
                st = {}
                items = []

                def mk(k):
                    def go():
                        if k == 0:
                            st["ps"] = psP.tile([128, 512], F32, tag="psP",
                                                name="ps_p")
                        Wt = WQs if is_q else WKs
                        nc.tensor.matmul(
                            st["ps"][:],
                            Wt[k][:, m * 128:(m + 1) * 128],
                            XT[k][:, n * 512:(n + 1) * 512],
                            start=(k == 0), stop=(k == KD - 1))
                    return go
                for k in range(KD):
                    items.append(mk(k))

                def cp():
                    dst = (QT if is_q else KT)[m][:, n * 512:(n + 1) * 512]
                    if qkv_bias:
                        bias_t = BQ if is_q else BK
                        nc.scalar.activation(dst, st["ps"][:], AF.Copy,
                                             bias=bias_t[:, m:m + 1])
                    else:
                        nc.vector.tensor_copy(dst, st["ps"][:])
                items.append(cp)
                return items

            def v_items(sc):
                st = {}
                items = []
                last_mm = KD - 1

                def mk(k):
                    def go():
                        if k == 0:
                            st["ps"] = psP.tile([128, 512], F32, tag="psP",
                                                name="ps_p")
                        nc.tensor.matmul(
                            st["ps"][:, 0:HE],
                            XT[k][:, sc * 128:(sc + 1) * 128],
                            WVs[k][:],
                            start=(k == 0),
                            stop=False if qkv_bias else (k == last_mm))
                    return go
                for k in range(KD):
                    items.append(mk(k))
                if qkv_bias:
                    items.append(lambda: nc.tensor.matmul(
                        st["ps"][:, 0:HE], ONES[:], BV[:],
                        start=False, stop=True))

                def cp():
                    nc.vector.tensor_copy(
                        VA[sc][:].rearrange("p (h c) -> p h c", c=65)[:, :, 0:64],
                        st["ps"][:, 0:HE].rearrange("p (h c) -> p h c", c=64))
                items.append(cp)
                return items

            def out_items(mc, half):
                st = {}
                items = []
                n0 = half * HE

                def mk(k):
                    def go():
                        if k == 0:
                            st["ps"] = psP.tile([128, 512], F32, tag="psP",
                                                name="ps_p")
                        nc.tensor.matmul(
                            st["ps"][:, 0:HE],
                            ZT[k][:, mc * 128:(mc + 1) * 128],
                            WO[k][:, n0:n0 + HE],
                            start=(k == 0), stop=(k == 2))
                    return go
                for k in range(3):
                    items.append(mk(k))

                def cp():
                    ob = workO.tile([128, HE], F32, tag="ob", name="ob")
                    nc.vector.tensor_copy(ob[:], st["ps"][:, 0:HE])
                    nc.sync.dma_start(
                        out=out[mc * 128:(mc + 1) * 128, n0:n0 + HE],
                        in_=ob[:])
                items.append(cp)
                return items

            def out_split_items(mc, half):
                st = {}
                n0 = half * HE

                def p0():
                    st["ps"] = psP.tile([128, 512], F32, tag="psP",
                                        name="ps_p")
                    nc.tensor.matmul(
                        st["ps"][:, 0:HE],
                        ZT[0][:, mc * 128:(mc + 1) * 128],
                        WO[0][:, n0:n0 + HE], start=True, stop=False)

                def p1():
                    nc.tensor.matmul(
                        st["ps"][:, 0:HE],
                        ZT[1][:, mc * 128:(mc + 1) * 128],
                        WO[1][:, n0:n0 + HE], start=False, stop=True)

                def pcp():
                    st["op"] = workO.tile([128, HE], F32, tag="op01",
                                          name="op01", bufs=8)
                    nc.vector.tensor_copy(st["op"][:], st["ps"][:, 0:HE])

                def f0():
                    st["ps2"] = psP.tile([128, 512], F32, tag="psP",
                                         name="ps_p")
                    nc.tensor.matmul(
                        st["ps2"][:, 0:HE],
                        ZT[2][:, mc * 128:(mc + 1) * 128],
                        WO[2][:, n0:n0 + HE], start=True, stop=True)

                def f1():
                    ob = workO.tile([128, HE], F32, tag="ob", name="ob")
                    nc.vector.tensor_add(ob[:], st["ps2"][:, 0:HE],
                                         st["op"][:])
                    nc.sync.dma_start(
                        out=out[mc * 128:(mc + 1) * 128, n0:n0 + HE],
                        in_=ob[:])
                return [p0, p1, pcp], [f0, f1]

            # Global interleave queue: (deadline, pred, fn).
            #  - deadline (hp, t, 0): the item WRITES a tile some score block
            #    of sweep (hp, t) reads, so it must be emitted before that
            #    sweep starts (emission-order WAR hazard otherwise).
            #    Enforced by drain() at every sweep start.
            #  - pred(pos) -> bool: earliest feed() may emit it.  Position
            #    gates reserve fillers for later, PE-thin phases; out-proj
            #    items are gated on the ZT writes they read being emitted
            #    (zt_ready counts normalize part2s per (hp, t)).
            # Q/K chunk n feeds sweep t=n of its pair; V[sc] feeds pair-0
            # sweep sc//4.
            GQ = []
            FAR = (9, 9, 9)
            zt_ready = {}

            def after(mark):
                return lambda pos: pos >= mark

            def zt_done(tq):
                return lambda pos: zt_ready.get((2, tq), 0) >= 2

            def put(deadline, pred, items):
                # one GQ entry per GROUP: the group's matmuls stay adjacent
                # in the PE stream (splitting them destroys LDWEIGHTS
                # pipelining); credit is spent per-item so pacing stays fine.
                GQ.append((deadline, pred, items, len(items)))

            always = after((-1, -1, -1))

            def sweep_before(m, n):
                # deadline one sweep EARLY: proj tiles must be written well
                # before the first score block that reads them, or the score
                # stalls on the projection's fresh psum->SBUF copy.
                return (m, n - 1, 0) if n >= 1 else (m - 1, 3, 0)

            # pair-0 remainder, ordered V-first within each deadline class
            for n in (2, 3):
                for sc in range(4 * n, 4 * n + 4):
                    put(sweep_before(0, n), always, v_items(sc))
                put(sweep_before(0, n), always, qk_items(True, 0, n))
                put(sweep_before(0, n), always, qk_items(False, 0, n))
            for n in range(4):
                put(sweep_before(1, n), always, qk_items(True, 1, n))
                put(sweep_before(1, n), always, qk_items(False, 1, n))
            # pair-2 projections: n0/n1 reserved for D(1), n2/n3 for D(2)'s
            # first two sweeps (which otherwise have no filler work).
            for n in range(4):
                pred = after((1, 0, 0)) if n < 2 else after((2, 0, 0))
                put(sweep_before(2, n), pred, qk_items(True, 2, n))
                put(sweep_before(2, n), pred, qk_items(False, 2, n))
            for tq in range(3):
                for mc in range(4 * tq, 4 * tq + 4):
                    for half in (0, 1):
                        put(FAR, zt_done(tq), out_items(mc, half))
            # tq=3 split: the ZT[0]/ZT[1] contributions run during sweep 3
            # (partial into SBUF); only the ZT[2] matmul + add + DMA wait for
            # the last normalize, shrinking the tail stall.
            _tq3 = [(mc, half) for mc in range(12, 16) for half in (0, 1)]
            _splits = {k: out_split_items(*k) for k in _tq3}
            for k in _tq3:
                put(FAR, after((2, 3, 0)), _splits[k][0])
            for k in _tq3:
                put(FAR, zt_done(3), _splits[k][1])

            feed_credit = [0.0]

            def run_group(entry):
                for it in entry[2]:
                    it()

            def feed(pos, r):
                feed_credit[0] += r
                while GQ and feed_credit[0] >= GQ[0][3] and GQ[0][1](pos):
                    e = GQ.pop(0)
                    run_group(e)
                    feed_credit[0] -= e[3]

            def drain(pos):
                # deadline-forced emission (sweep-start prerequisites)
                while GQ and GQ[0][0] <= pos:
                    run_group(GQ.pop(0))

            def drain_all():
                while GQ:
                    run_group(GQ.pop(0))

            # ---------------- attention stream ----------------
            # av_q is GLOBAL: the deferred-AV lag flows across pair
            # boundaries, so a new pair's (independent) score blocks and
            # sweep-start filler bursts run while the old pair's last exps
            # finish, instead of the PE head-of-line blocking on them.
            av_q = []      # aged deferred work: (fn, norm_fn | None)

            def pump_avs(lag):
                while len(av_q) > lag:
                    av_fn, norm_fn = av_q.pop(0)
                    av_fn()
                    if norm_fn is not None:
                        norm_fn()

            def run_pair(hp, rate):
                psz = {}

                def emit_normalize(par, t):
                    ho = par * 64
                    # drain psz out of PSUM fast (frees the bank)
                    zraw = workZ.tile([65, 512], BF16, tag="zraw", name="zraw")
                    nc.vector.tensor_copy(zraw[:], psz[par][:])

                    # reciprocal of the denominator row, reshaped across all
                    # 128 DVE lanes via a direct SBUF->SBUF strided DMA
                    # ([1,512] row -> [128,4]); a 1-lane [1,512] reciprocal
                    # costs 3.3us.
                    zr = zraw[64:65, :]
                    rp = work2.tile([128, 4], BF16, tag="rp", name="rp")
                    nc.sync.dma_start(out=rp[:], in_=bass.AP(
                        tensor=zr.tensor, offset=zr.offset,
                        ap=[zr.ap[0], [4, 128], [1, 4]]))
                    rcp = work2.tile([128, 4], BF16, tag="rcp", name="rcp")
                    with nc.allow_low_precision(reason="softmax recip bf16"):
                        nc.vector.reciprocal(rcp[:], rp[:])
                    rcd = dramP.tile([1, 512], BF16, tag="rcd", name="rcd")
                    rcd_ap = rcd[:]
                    nc.sync.dma_start(out=bass.AP(
                        tensor=rcd_ap.tensor, offset=rcd_ap.offset,
                        ap=[[4, 128], [1, 4]]), in_=rcp[:])
                    bc = work2.tile([64, 512], BF16, tag="bc", name="bc")
                    nc.sync.dma_start(out=bc[:], in_=bass.AP(
                        tensor=rcd_ap.tensor, offset=rcd_ap.offset,
                        ap=[[0, 64], rcd_ap.ap[-1]]))

                    def part2():
                        # final scale, re-queued behind AV_LAG more items so
                        # the reciprocal's DMA-bounce latency is hidden
                        nc.vector.tensor_mul(
                            ZT[hp][ho:ho + 64, t * 512:(t + 1) * 512],
                            zraw[0:64, :], bc[:])
                        zt_ready[(hp, t)] = zt_ready.get((hp, t), 0) + 1
                    av_q.append((part2, None))

                for t in range(4):
                    drain((hp, t, 0))
                    for j in range(4 * t + 4):
                        r = j - 4 * t  # >= 0 only on the diagonal
                        q0 = 128 * r if r >= 0 else 0
                        # Both heads' scores in ONE psum tile, anchored at the
                        # bank boundary: par0 writes [q0, 512) (bank 0), par1
                        # [512, 512+w) (bank 1).  The two matmuls run
                        # concurrently (disjoint row groups) so they must hit
                        # DIFFERENT single-port psum banks, and the regions
                        # stay contiguous so one exp instruction covers both.
                        w = 512 - q0
                        c0s = (q0, 512)
                        pss = psS.tile([128, 1024], F32, tag="pss", name="pss")
                        for par in (0, 1):
                            ho = par * 64
                            nc.tensor.matmul(
                                pss[:, c0s[par]:c0s[par] + w],
                                KT[hp][ho:ho + 64, j * 128:(j + 1) * 128],
                                QT[hp][ho:ho + 64, t * 512 + q0:(t + 1) * 512],
                                start=True, stop=True)
                        et = workE.tile([128, 1024], BF16, tag="et", name="et")
                        nc.scalar.activation(et[:, q0:512 + w],
                                             pss[:, q0:512 + w],
                                             AF.Exp, scale=0.125)
                        if r >= 0:
                            # gpsimd (otherwise idle): keeps the DVE queue
                            # short so AV-gating work isn't behind proj copies
                            for c0 in c0s:
                                nc.gpsimd.tensor_mul(
                                    et[:, c0:c0 + 128],
                                    et[:, c0:c0 + 128],
                                    MSK[:])
                        for par in (0, 1):
                            a0 = c0s[par]  # valid region start in et

                            def av_fn(par=par, et=et, j=j, t=t, a0=a0, q0=q0):
                                if j == 0:
                                    psz[par] = psZ.tile(
                                        [65, 512], F32, tag=f"psz{par}",
                                        name=f"psz{par}")
                                h = 2 * hp + par
                                nc.tensor.matmul(
                                    psz[par][:, q0:512],
                                    VA[j][:, h * 65:(h + 1) * 65],
                                    et[:, a0:a0 + 512 - q0],
                                    start=(j == 0), stop=(j == 4 * t + 3))
                            norm_fn = (
                                lambda par=par, t=t:
                                emit_normalize(par, t)) if j == 4 * t + 3 else None
                            av_q.append((av_fn, norm_fn))
                        feed((hp, t, j), rate)
                        pump_avs(AV_LAG)
                    pump_avs(2)

            # upfront projections for pair 0's first two sweeps (PE-only
            # prologue; everything else interleaves into the D streams).
            for n in (0, 1):
                for it in qk_items(True, 0, n) + qk_items(False, 0, n):
                    it()
            for sc in range(8):
                for it in v_items(sc):
                    it()

            run_pair(0, 2.6)
            run_pair(1, 2.2)
            run_pair(2, 3.2)
            pump_avs(0)
            drain_all()
    nc.compile()
    return nc


def _build_mask() -> np.ndarray:
    # triangle for the strict-diagonal 128x128 strip: 1.0 iff q_local >= k_local
    kl = np.arange(128)[:, None]
    ql = np.arange(128)[None, :]
    return (ql >= kl).astype(np.float32)


def kernel(**inputs) -> np.ndarray:
    global LAST_EXEC_TIME_NS
    x = np.asarray(inputs["normalized_resid_pre"], dtype=np.float32)
    W_Q = np.asarray(inputs["W_Q"], dtype=np.float32)
    W_K = np.asarray(inputs["W_K"], dtype=np.float32)
    W_V = np.asarray(inputs["W_V"], dtype=np.float32)
    W_O = np.asarray(inputs["W_O"], dtype=np.float32)
    b_Q = np.asarray(inputs["b_Q"], dtype=np.float32)
    b_K = np.asarray(inputs["b_K"], dtype=np.float32)
    b_V = np.asarray(inputs["b_V"], dtype=np.float32)
    b_O = np.asarray(inputs["b_O"], dtype=np.float32)

    qkv_bias = bool(b_Q.any() or b_K.any() or b_V.any())
    key = qkv_bias
    if key not in _GRAPH_CACHE:
        _GRAPH_CACHE[key] = _build_graph(qkv_bias)
    nc = _GRAPH_CACHE[key]

    mask = _build_mask()
    in_maps = []
    for c in range(8):
        b, h0 = c // 2, NHC * (c % 2)
        im = {
            "xt": np.ascontiguousarray(x[b].T).astype(BF16NP),
            "wq": np.ascontiguousarray(
                W_Q[h0:h0 + NHC].transpose(1, 0, 2).reshape(D, HE)).astype(BF16NP),
            "wk": np.ascontiguousarray(
                W_K[h0:h0 + NHC].transpose(1, 0, 2).reshape(D, HE)).astype(BF16NP),
            "wv": np.ascontiguousarray(
                W_V[h0:h0 + NHC].transpose(1, 0, 2).reshape(D, HE)).astype(BF16NP),
            "wo": np.ascontiguousarray(W_O[h0:h0 + NHC].reshape(HE, D)).astype(BF16NP),
            "mask": mask.astype(BF16NP),
        }
        if qkv_bias:
            im["bq"] = np.ascontiguousarray(b_Q[h0:h0 + NHC].reshape(HE, 1))
            im["bk"] = np.ascontiguousarray(b_K[h0:h0 + NHC].reshape(HE, 1))
            im["bv"] = np.ascontiguousarray(b_V[h0:h0 + NHC].reshape(1, HE)).astype(BF16NP)
        in_maps.append(im)

    import os
    trace = bool(os.environ.get("KERNEL_TRACE"))
    res = run_bass_kernel_spmd(nc, in_maps, core_ids=list(range(8)), trace=trace)
    LAST_EXEC_TIME_NS = res.exec_time_ns
    results = res.results

    out = np.empty((B, S, D), dtype=np.float32)
    for b in range(B):
        out[b] = results[2 * b]["out"] + results[2 * b + 1]["out"]
    if b_O.any():
        out += b_O
    return out


# revision 46
# speedup vs baseline: 1.2566x; 1.0054x over previous
"""Causal multi-head attention on 8 TRN2 NeuronCores.

Problem: x[4, 2048, 768], 12 heads x d_head 64, causal softmax attention.

Sharding: core c handles batch b = c//2 and the 6-head group h0 = 6*(c%2).
Each core computes its partial output o_partial[2048, 768] = sum over its 6
heads of (softmax(QK^T/8) V) @ W_O.  The two cores sharing a batch are summed
on the host (part of unsharding), so the device graph needs no collectives.

Device layout (per core) avoids every on-chip transpose:
  - host passes x^T  [768, 2048]  (xt)
  - Q^T, K^T [384, 2048] = W^T @ x^T   (lhsT = W slice, rhs = x^T)
  - V        [2048, 384]  = x @ W_V    (lhsT = x^T slice, rhs = W_V)
  - scores^T blocks [128k, 512q] = (K^T slice).T @ Q^T slice  (per head, K=64)
  - exp on ACT (scale=1/8 folded in); causal mask = 0/1 multiply on diagonal
    blocks; V is stored augmented with a ones column per head so the single
    AV matmul produces both z^T rows (64) and the softmax denominators (row 64)
  - z^T normalized via DMA-broadcast reciprocal row, stored as ZT [384, 2048]
  - out = (ZT).T @ W_O   (lhsT = ZT slice, rhs = W_O)
All matmuls run in bf16 (1 cycle/column vs 4 for fp32; f32 PSUM accum).

Scheduling: the attention phase is ACT(exp)-bound (~35us of exp per head
pair vs ~25us of PE), while projections and the output projection are
PE-only.  So emission interleaves them: while pair hp's attention stream
keeps ACT busy, the PE also runs pair hp+1's Q/K projections (and, during
the last pair, the output-projection tiles), paced by a credit-fed global
queue with per-sweep deadlines (emission-order WAR safety) and eligibility
gates (fillers reserved for PE-thin phases; out-proj gated on the ZT writes
it reads being emitted).  q-supertiles are processed one at a time (not in
halves) so PSUM fits scores (4 banks) + z accumulators (2) + a
projection/output slot (2).  Every score block writes the two heads'
columns anchored at the psum bank boundary (par0 [q0,512) in bank 0, par1
[512,512+w) in bank 1): the concurrent matmuls hit different single-port
banks, and one exp instruction covers both heads.  Deferred AV / normalize
closures ride a global lag queue that flows across sweep and pair
boundaries; causal-mask multiplies run on the otherwise-idle GpSimd engine;
the last supertile's output projection is split so only the ZT[2] matmul
waits on the final normalize.

Note: the chip drops the PE from 2.4 to ~2.0 GHz (P0 power state) under
sustained load, so back-to-back benchmark runs measure ~10-13% slower than
cooled runs (~236us cooled, ~268us hot).
"""

import sys

if "/opt/trn_rl_repo" not in sys.path:
    sys.path.insert(0, "/opt/trn_rl_repo")

import numpy as np
import ml_dtypes

BF16NP = ml_dtypes.bfloat16


def _ensure_ntff_hook():
    """The agent image's `antenv` lacks `axon_hooks`, which bass_utils needs
    for trace=True under axon. Recreate it via sys.modules injection using the
    boot helper's ctypes wrapper around libaxon_pjrt.so."""
    import types
    if "antenv.axon_hooks" in sys.modules:
        return
    try:
        from trn_agent_boot.trn_boot import _ntff_profile_via_ctypes
        hook = _ntff_profile_via_ctypes("/opt/axon/libaxon_pjrt.so")
    except Exception:
        hook = None
    m = types.ModuleType("antenv.axon_hooks")
    m._hook = hook
    m.get_axon_ntff_profile_hook = lambda: m._hook
    def _set(h):
        m._hook = h
    m.set_axon_ntff_profile_hook = _set
    sys.modules["antenv.axon_hooks"] = m


_ensure_ntff_hook()

import concourse.bass as bass
import concourse.tile as tile
from concourse import bacc, mybir
from concourse.bass_utils import run_bass_kernel_spmd

F32 = mybir.dt.float32
BF16 = mybir.dt.bfloat16
AF = mybir.ActivationFunctionType

D = 768          # d_model
S = 2048         # seq
E = 64           # d_head
NHC = 6          # heads per core
HE = NHC * E     # 384
KD = D // 128    # 6 k-chunks over d_model
B = 4

AV_LAG = 8       # deferred-AV depth (keeps PE from stalling on exp)

LAST_EXEC_TIME_NS = None
_GRAPH_CACHE = {}


def _build_graph(qkv_bias: bool) -> bass.Bass:
    nc = bacc.Bacc("TRN2", target_bir_lowering=False)
    xt = nc.declare_dram_parameter("xt", [D, S], BF16, isOutput=False)
    wq = nc.declare_dram_parameter("wq", [D, HE], BF16, isOutput=False)
    wk = nc.declare_dram_parameter("wk", [D, HE], BF16, isOutput=False)
    wv = nc.declare_dram_parameter("wv", [D, HE], BF16, isOutput=False)
    wo = nc.declare_dram_parameter("wo", [HE, D], BF16, isOutput=False)
    mask = nc.declare_dram_parameter("mask", [128, 128], BF16, isOutput=False)
    if qkv_bias:
        bq = nc.declare_dram_parameter("bq", [HE, 1], F32, isOutput=False)
        bk = nc.declare_dram_parameter("bk", [HE, 1], F32, isOutput=False)
        bv = nc.declare_dram_parameter("bv", [1, HE], BF16, isOutput=False)
    out = nc.declare_dram_parameter("out", [S, D], F32, isOutput=True)

    with tile.TileContext(nc) as tc:
        with tc.tile_pool(name="persist", bufs=1) as persist, \
             tc.tile_pool(name="workE", bufs=14) as workE, \
             tc.tile_pool(name="workZ", bufs=8) as workZ, \
             tc.tile_pool(name="work2", bufs=4) as work2, \
             tc.tile_pool(name="workO", bufs=4) as workO, \
             tc.tile_pool(name="dramP", bufs=3, space="DRAM") as dramP, \
             tc.tile_pool(name="psP", bufs=2, space="PSUM") as psP, \
             tc.tile_pool(name="psS", bufs=2, space="PSUM") as psS, \
             tc.tile_pool(name="psZ", bufs=1, space="PSUM") as psZ:

            QT = [persist.tile([128, S], BF16, tag=f"qt{m}", name=f"qt{m}") for m in range(3)]
            KT = [persist.tile([128, S], BF16, tag=f"kt{m}", name=f"kt{m}") for m in range(3)]
            ZT = [persist.tile([128, S], BF16, tag=f"zt{m}", name=f"zt{m}") for m in range(3)]
            VA = [persist.tile([128, NHC * 65], BF16, tag=f"va{s}", name=f"va{s}") for s in range(16)]
            WO = [persist.tile([128, D], BF16, tag=f"wo{m}", name=f"wo{m}") for m in range(3)]
            MSK = persist.tile([128, 128], BF16, tag="mask", name="mask_sb")
            XT = [persist.tile([128, S], BF16, tag=f"xt{k}", name=f"xt{k}") for k in range(KD)]
            WQs = [persist.tile([128, HE], BF16, tag=f"wq{k}", name=f"wq{k}") for k in range(KD)]
            WKs = [persist.tile([128, HE], BF16, tag=f"wk{k}", name=f"wk{k}") for k in range(KD)]
            WVs = [persist.tile([128, HE], BF16, tag=f"wv{k}", name=f"wv{k}") for k in range(KD)]

            # issue order matters: the Sync engine issues dma_starts serially
            # (~340ns each), so the first Q/K projection group's inputs
            # (XT[k], WQs[k], WKs[k]) go first, in consumption order.
            for k in range(KD):
                nc.sync.dma_start(out=XT[k][:], in_=xt[k * 128:(k + 1) * 128, :])
                nc.sync.dma_start(out=WQs[k][:], in_=wq[k * 128:(k + 1) * 128, :])
                nc.sync.dma_start(out=WKs[k][:], in_=wk[k * 128:(k + 1) * 128, :])
            for k in range(KD):
                nc.sync.dma_start(out=WVs[k][:], in_=wv[k * 128:(k + 1) * 128, :])
            for m in range(3):
                nc.sync.dma_start(out=WO[m][:], in_=wo[m * 128:(m + 1) * 128, :])
            nc.sync.dma_start(out=MSK[:], in_=mask[:])
            ONES = persist.tile([1, 128], BF16, tag="ones", name="ones_sb")
            nc.vector.memset(ONES[:], 1.0)
            DUM = persist.tile([128, 512], BF16, tag="dum", name="dum_sb")
            nc.vector.memset(DUM[:], 1.0)

            def warm(n):
                # dependency-free matmuls: keep the PE busy (and the HAM
                # clock un-throttled) across stalls the scheduler can't fill
                # -- startup DMA waits and the tail normalize chain.  K must
                # be 128 (full rows): a thin-K stream reads as LOW activity
                # and actively cools the clock.
                for _ in range(n):
                    d = psP.tile([128, 512], F32, tag="psP", name="ps_warm")
                    nc.tensor.matmul(d[:], DUM[:, 0:128], DUM[:],
                                     start=True, stop=True)
            if qkv_bias:
                BQ = persist.tile([128, 3], F32, tag="bq", name="bq_sb")
                BK = persist.tile([128, 3], F32, tag="bk", name="bk_sb")
                BV = persist.tile([1, HE], BF16, tag="bv", name="bv_sb")
                for m in range(3):
                    nc.sync.dma_start(out=BQ[:, m:m + 1], in_=bq[m * 128:(m + 1) * 128, :])
                    nc.sync.dma_start(out=BK[:, m:m + 1], in_=bk[m * 128:(m + 1) * 128, :])
                nc.sync.dma_start(out=BV[:], in_=bv[:])
            for sc in range(16):
                nc.vector.memset(VA[sc][:], 1.0)

            # ---------------- interleave items ----------------
            # Filler work is split to single-matmul granularity (~200ns per
            # item) so feed() can smooth the PE load between attention blocks
            # -- chunky fillers leave micro-gaps that re-throttle the HAM
            # clock.  Items of one accumulation group share the psum tile via
            # a closure dict and stay adjacent in GQ order.
            def qk_items(is_q, m, n):
                st = {}
                items = []

                def mk(k):
                    def go():
                        if k == 0:
                            st["ps"] = psP.tile([128, 512], F32, tag="psP",
                                                name="ps_p")
                        Wt = WQs if is_q else WKs
                        nc.tensor.matmul(
                            st["ps"][:],
                            Wt[k][:, m * 128:(m + 1) * 128],
                            XT[k][:, n * 512:(n + 1) * 512],
                            start=(k == 0), stop=(k == KD - 1))
                    return go
                for k in range(KD):
                    items.append(mk(k))

                def cp():
                    dst = (QT if is_q else KT)[m][:, n * 512:(n + 1) * 512]
                    if qkv_bias:
                        bias_t = BQ if is_q else BK
                        nc.scalar.activation(dst, st["ps"][:], AF.Copy,
                                             bias=bias_t[:, m:m + 1])
                    else:
                        nc.vector.tensor_copy(dst, st["ps"][:])
                items.append(cp)
                return items

            def v_items(sc):
                st = {}
                items = []
                last_mm = KD - 1

                def mk(k):
                    def go():
                        if k == 0:
                            st["ps"] = psP.tile([128, 512], F32, tag="psP",
                                                name="ps_p")
                        nc.tensor.matmul(
                            st["ps"][:, 0:HE],
                            XT[k][:, sc * 128:(sc + 1) * 128],
                            WVs[k][:],
                            start=(k == 0),
                            stop=False if qkv_bias else (k == last_mm))
                    return go
                for k in range(KD):
                    items.append(mk(k))
                if qkv_bias:
                    items.append(lambda: nc.tensor.matmul(
                        st["ps"][:, 0:HE], ONES[:], BV[:],
                        start=False, stop=True))

                def cp():
                    nc.vector.tensor_copy(
                        VA[sc][:].rearrange("p (h c) -> p h c", c=65)[:, :, 0:64],
                        st["ps"][:, 0:HE].rearrange("p (h c) -> p h c", c=64))
                items.append(cp)
                return items

            def out_items(mc, half):
                st = {}
                items = []
                n0 = half * HE

                def mk(k):
                    def go():
                        if k == 0:
                            st["ps"] = psP.tile([128, 512], F32, tag="psP",
                                                name="ps_p")
                        nc.tensor.matmul(
                            st["ps"][:, 0:HE],
                            ZT[k][:, mc * 128:(mc + 1) * 128],
                            WO[k][:, n0:n0 + HE],
                            start=(k == 0), stop=(k == 2))
                    return go
                for k in range(3):
                    items.append(mk(k))

                def cp():
                    ob = workO.tile([128, HE], F32, tag="ob", name="ob")
                    nc.vector.tensor_copy(ob[:], st["ps"][:, 0:HE])
                    nc.sync.dma_start(
                        out=out[mc * 128:(mc + 1) * 128, n0:n0 + HE],
                        in_=ob[:])
                items.append(cp)
                return items

            def out_split_items(mc, half):
                st = {}
                n0 = half * HE

                def p0():
                    st["ps"] = psP.tile([128, 512], F32, tag="psP",
                                        name="ps_p")
                    nc.tensor.matmul(
                        st["ps"][:, 0:HE],
                        ZT[0][:, mc * 128:(mc + 1) * 128],
                        WO[0][:, n0:n0 + HE], start=True, stop=False)

                def p1():
                    nc.tensor.matmul(
                        st["ps"][:, 0:HE],
                        ZT[1][:, mc * 128:(mc + 1) * 128],
                        WO[1][:, n0:n0 + HE], start=False, stop=True)

                def pcp():
                    st["op"] = workO.tile([128, HE], F32, tag="op01",
                                          name="op01", bufs=8)
                    nc.vector.tensor_copy(st["op"][:], st["ps"][:, 0:HE])

                def f0():
                    # ZT[2]'s last supertile holds RAW z: run the two heads
                    # as concurrent K=64 matmuls (disjoint row groups, two
                    # psum banks) and normalize in the combine below.
                    st["psA"] = psP.tile([128, 512], F32, tag="psP",
                                         name="ps_p")
                    nc.tensor.matmul(
                        st["psA"][:, 0:HE],
                        ZT[2][0:64, mc * 128:(mc + 1) * 128],
                        WO[2][0:64, n0:n0 + HE], start=True, stop=True)
                    st["psB"] = psP.tile([128, 512], F32, tag="psP",
                                         name="ps_p")
                    nc.tensor.matmul(
                        st["psB"][:, 0:HE],
                        ZT[2][64:128, mc * 128:(mc + 1) * 128],
                        WO[2][64:128, n0:n0 + HE], start=True, stop=True)

                def f1():
                    # ob = psA/denomA + psB/denomB + partial01, with the
                    # reciprocals as per-partition scalars (RC4[par][:, cc])
                    cc = mc - 12
                    mult = mybir.AluOpType.mult
                    add = mybir.AluOpType.add
                    ob = workO.tile([128, HE], F32, tag="ob", name="ob")
                    nc.vector.scalar_tensor_tensor(
                        ob[:], st["psA"][:, 0:HE], RC4[0][:, cc:cc + 1],
                        st["op"][:], mult, add)
                    nc.vector.scalar_tensor_tensor(
                        ob[:], st["psB"][:, 0:HE], RC4[1][:, cc:cc + 1],
                        ob[:], mult, add)
                    nc.sync.dma_start(
                        out=out[mc * 128:(mc + 1) * 128, n0:n0 + HE],
                        in_=ob[:])
                return [p0, p1, pcp], [f0, f1]

            # Global interleave queue: (deadline, pred, fn).
            #  - deadline (hp, t, 0): the item WRITES a tile some score block
            #    of sweep (hp, t) reads, so it must be emitted before that
            #    sweep starts (emission-order WAR hazard otherwise).
            #    Enforced by drain() at every sweep start.
            #  - pred(pos) -> bool: earliest feed() may emit it.  Position
            #    gates reserve fillers for later, PE-thin phases; out-proj
            #    items are gated on the ZT writes they read being emitted
            #    (zt_ready counts normalize part2s per (hp, t)).
            # Q/K chunk n feeds sweep t=n of its pair; V[sc] feeds pair-0
            # sweep sc//4.
            GQ = []
            FAR = (9, 9, 9)
            zt_ready = {}
            RC4 = {}   # par -> [128,4] reciprocal cols for the raw-z tail

            def after(mark):
                return lambda pos: pos >= mark

            def zt_done(tq):
                return lambda pos: zt_ready.get((2, tq), 0) >= 2

            def put(deadline, pred, items):
                # one GQ entry per GROUP: the group's matmuls stay adjacent
                # in the PE stream (splitting them destroys LDWEIGHTS
                # pipelining); credit is spent per-item so pacing stays fine.
                GQ.append((deadline, pred, items, len(items)))

            always = after((-1, -1, -1))

            def sweep_before(m, n):
                # deadline one sweep EARLY: proj tiles must be written well
                # before the first score block that reads them, or the score
                # stalls on the projection's fresh psum->SBUF copy.
                return (m, n - 1, 0) if n >= 1 else (m - 1, 3, 0)

            # pair-0 remainder, ordered V-first within each deadline class
            for n in (2, 3):
                for sc in range(4 * n, 4 * n + 4):
                    put(sweep_before(0, n), always, v_items(sc))
                put(sweep_before(0, n), always, qk_items(True, 0, n))
                put(sweep_before(0, n), always, qk_items(False, 0, n))
            for n in range(4):
                put(sweep_before(1, n), always, qk_items(True, 1, n))
                put(sweep_before(1, n), always, qk_items(False, 1, n))
            # pair-2 projections: n0/n1 reserved for D(1), n2/n3 for D(2)'s
            # first two sweeps (which otherwise have no filler work).
            for n in range(4):
                pred = after((1, 0, 0)) if n < 2 else after((2, 0, 0))
                put(sweep_before(2, n), pred, qk_items(True, 2, n))
                put(sweep_before(2, n), pred, qk_items(False, 2, n))
            for tq in range(3):
                for mc in range(4 * tq, 4 * tq + 4):
                    for half in (0, 1):
                        put(FAR, zt_done(tq), out_items(mc, half))
            # tq=3 split: the ZT[0]/ZT[1] contributions run during sweep 3
            # (partial into SBUF); only the ZT[2] matmul + add + DMA wait for
            # the last normalize, shrinking the tail stall.
            _tq3 = [(mc, half) for mc in range(12, 16) for half in (0, 1)]
            _splits = {k: out_split_items(*k) for k in _tq3}
            for k in _tq3:
                put(FAR, after((2, 3, 0)), _splits[k][0])
            for k in _tq3:
                put(FAR, zt_done(3), _splits[k][1])

            feed_credit = [0.0]

            def run_group(entry):
                for it in entry[2]:
                    it()

            def feed(pos, r):
                feed_credit[0] += r
                while GQ and feed_credit[0] >= GQ[0][3] and GQ[0][1](pos):
                    e = GQ.pop(0)
                    run_group(e)
                    feed_credit[0] -= e[3]

            def drain(pos):
                # deadline-forced emission (sweep-start prerequisites)
                while GQ and GQ[0][0] <= pos:
                    run_group(GQ.pop(0))

            def drain_all():
                while GQ:
                    run_group(GQ.pop(0))

            # ---------------- attention stream ----------------
            # av_q is GLOBAL: the deferred-AV lag flows across pair
            # boundaries, so a new pair's (independent) score blocks and
            # sweep-start filler bursts run while the old pair's last exps
            # finish, instead of the PE head-of-line blocking on them.
            av_q = []      # aged deferred work: (fn, norm_fn | None)

            def pump_avs(lag):
                while len(av_q) > lag:
                    av_fn, norm_fn = av_q.pop(0)
                    av_fn()
                    if norm_fn is not None:
                        norm_fn()

            def run_pair(hp, rate):
                psz = {}

                def emit_normalize(par, t):
                    ho = par * 64
                    # drain psz out of PSUM fast (frees the bank)
                    zraw = workZ.tile([65, 512], BF16, tag="zraw", name="zraw")
                    nc.vector.tensor_copy(zraw[:], psz[par][:])
                    zr = zraw[64:65, :]

                    if hp == 2 and t == 3:
                        # Final supertile: skip the broadcast-normalize whose
                        # DMA latency would sit exposed at the kernel tail.
                        # ZT gets RAW z; the reciprocal lands as a [128, 4]
                        # per-partition column (partition = q-local, column =
                        # mc chunk) and is applied as a tensor_scalar operand
                        # in the output combine, which overlaps the bounce.
                        dd = dramP.tile([1, 512], BF16, tag=f"dd{par}",
                                        name=f"dd{par}")
                        nc.sync.dma_start(out=dd[:], in_=zr)
                        dd_ap = dd[:]
                        rp4 = work2.tile([128, 4], BF16, tag=f"rp4{par}",
                                         name=f"rp4{par}")
                        nc.sync.dma_start(out=rp4[:], in_=bass.AP(
                            tensor=dd_ap.tensor, offset=dd_ap.offset,
                            ap=[[1, 128], [128, 4]]))
                        rc4 = work2.tile([128, 4], BF16, tag=f"rc4{par}",
                                         name=f"rc4{par}")
                        with nc.allow_low_precision(reason="softmax recip"):
                            nc.vector.reciprocal(rc4[:], rp4[:])
                        RC4[par] = rc4

                        def part2r():
                            nc.vector.tensor_copy(
                                ZT[hp][ho:ho + 64, t * 512:(t + 1) * 512],
                                zraw[0:64, :])
                            zt_ready[(hp, t)] = zt_ready.get((hp, t), 0) + 1
                        av_q.append((part2r, None))
                        return

                    # reciprocal of the denominator row, reshaped across all
                    # 128 DVE lanes via a direct SBUF->SBUF strided DMA
                    # ([1,512] row -> [128,4]); a 1-lane [1,512] reciprocal
                    # costs 3.3us.
                    rp = work2.tile([128, 4], BF16, tag="rp", name="rp")
                    nc.sync.dma_start(out=rp[:], in_=bass.AP(
                        tensor=zr.tensor, offset=zr.offset,
                        ap=[zr.ap[0], [4, 128], [1, 4]]))
                    rcp = work2.tile([128, 4], BF16, tag="rcp", name="rcp")
                    with nc.allow_low_precision(reason="softmax recip bf16"):
                        nc.vector.reciprocal(rcp[:], rp[:])
                    rcd = dramP.tile([1, 512], BF16, tag="rcd", name="rcd")
                    rcd_ap = rcd[:]
                    nc.sync.dma_start(out=bass.AP(
                        tensor=rcd_ap.tensor, offset=rcd_ap.offset,
                        ap=[[4, 128], [1, 4]]), in_=rcp[:])
                    bc = work2.tile([64, 512], BF16, tag="bc", name="bc")
                    nc.sync.dma_start(out=bc[:], in_=bass.AP(
                        tensor=rcd_ap.tensor, offset=rcd_ap.offset,
                        ap=[[0, 64], rcd_ap.ap[-1]]))

                    def part2():
                        # final scale, re-queued behind AV_LAG more items so
                        # the reciprocal's DMA-bounce latency is hidden
                        nc.vector.tensor_mul(
                            ZT[hp][ho:ho + 64, t * 512:(t + 1) * 512],
                            zraw[0:64, :], bc[:])
                        zt_ready[(hp, t)] = zt_ready.get((hp, t), 0) + 1
                    av_q.append((part2, None))

                for t in range(4):
                    drain((hp, t, 0))
                    for j in range(4 * t + 4):
                        r = j - 4 * t  # >= 0 only on the diagonal
                        q0 = 128 * r if r >= 0 else 0
                        # Both heads' scores in ONE psum tile, anchored at the
                        # bank boundary: par0 writes [q0, 512) (bank 0), par1
                        # [512, 512+w) (bank 1).  The two matmuls run
                        # concurrently (disjoint row groups) so they must hit
                        # DIFFERENT single-port psum banks, and the regions
                        # stay contiguous so one exp instruction covers both.
                        w = 512 - q0
                        c0s = (q0, 512)
                        pss = psS.tile([128, 1024], F32, tag="pss", name="pss")
                        for par in (0, 1):
                            ho = par * 64
                            nc.tensor.matmul(
                                pss[:, c0s[par]:c0s[par] + w],
                                KT[hp][ho:ho + 64, j * 128:(j + 1) * 128],
                                QT[hp][ho:ho + 64, t * 512 + q0:(t + 1) * 512],
                                start=True, stop=True)
                        et = workE.tile([128, 1024], BF16, tag="et", name="et")
                        nc.scalar.activation(et[:, q0:512 + w],
                                             pss[:, q0:512 + w],
                                             AF.Exp, scale=0.125)
                        if r >= 0:
                            # gpsimd (otherwise idle): keeps the DVE queue
                            # short so AV-gating work isn't behind proj copies
                            for c0 in c0s:
                                nc.gpsimd.tensor_mul(
                                    et[:, c0:c0 + 128],
                                    et[:, c0:c0 + 128],
                                    MSK[:])
                        for par in (0, 1):
                            a0 = c0s[par]  # valid region start in et

                            def av_fn(par=par, et=et, j=j, t=t, a0=a0, q0=q0):
                                if j == 0:
                                    psz[par] = psZ.tile(
                                        [65, 512], F32, tag=f"psz{par}",
                                        name=f"psz{par}")
                                h = 2 * hp + par
                                nc.tensor.matmul(
                                    psz[par][:, q0:512],
                                    VA[j][:, h * 65:(h + 1) * 65],
                                    et[:, a0:a0 + 512 - q0],
                                    start=(j == 0), stop=(j == 4 * t + 3))
                            norm_fn = (
                                lambda par=par, t=t:
                                emit_normalize(par, t)) if j == 4 * t + 3 else None
                            av_q.append((av_fn, norm_fn))
                        feed((hp, t, j), rate)
                        pump_avs(AV_LAG)
                    pump_avs(2)

            # upfront projections for pair 0's first two sweeps (PE-only
            # prologue; everything else interleaves into the D streams).
            for n in (0, 1):
                for it in qk_items(True, 0, n) + qk_items(False, 0, n):
                    it()
            for sc in range(8):
                for it in v_items(sc):
                    it()

            run_pair(0, 2.6)
            run_pair(1, 2.2)
            run_pair(2, 3.2)
            pump_avs(0)
            drain_all()
    nc.compile()
    return nc


def _build_mask() -> np.ndarray:
    # triangle for the strict-diagonal 128x128 strip: 1.0 iff q_local >= k_local
    kl = np.arange(128)[:, None]
    ql = np.arange(128)[None, :]
    return (ql >= kl).astype(np.float32)


def kernel(**inputs) -> np.ndarray:
    global LAST_EXEC_TIME_NS
    x = np.asarray(inputs["normalized_resid_pre"], dtype=np.float32)
    W_Q = np.asarray(inputs["W_Q"], dtype=np.float32)
    W_K = np.asarray(inputs["W_K"], dtype=np.float32)
    W_V = np.asarray(inputs["W_V"], dtype=np.float32)
    W_O = np.asarray(inputs["W_O"], dtype=np.float32)
    b_Q = np.asarray(inputs["b_Q"], dtype=np.float32)
    b_K = np.asarray(inputs["b_K"], dtype=np.float32)
    b_V = np.asarray(inputs["b_V"], dtype=np.float32)
    b_O = np.asarray(inputs["b_O"], dtype=np.float32)

    qkv_bias = bool(b_Q.any() or b_K.any() or b_V.any())
    key = qkv_bias
    if key not in _GRAPH_CACHE:
        _GRAPH_CACHE[key] = _build_graph(qkv_bias)
    nc = _GRAPH_CACHE[key]

    mask = _build_mask()
    in_maps = []
    for c in range(8):
        b, h0 = c // 2, NHC * (c % 2)
        im = {
            "xt": np.ascontiguousarray(x[b].T).astype(BF16NP),
            "wq": np.ascontiguousarray(
                W_Q[h0:h0 + NHC].transpose(1, 0, 2).reshape(D, HE)).astype(BF16NP),
            "wk": np.ascontiguousarray(
                W_K[h0:h0 + NHC].transpose(1, 0, 2).reshape(D, HE)).astype(BF16NP),
            "wv": np.ascontiguousarray(
                W_V[h0:h0 + NHC].transpose(1, 0, 2).reshape(D, HE)).astype(BF16NP),
            "wo": np.ascontiguousarray(W_O[h0:h0 + NHC].reshape(HE, D)).astype(BF16NP),
            "mask": mask.astype(BF16NP),
        }
        if qkv_bias:
            im["bq"] = np.ascontiguousarray(b_Q[h0:h0 + NHC].reshape(HE, 1))
            im["bk"] = np.ascontiguousarray(b_K[h0:h0 + NHC].reshape(HE, 1))
            im["bv"] = np.ascontiguousarray(b_V[h0:h0 + NHC].reshape(1, HE)).astype(BF16NP)
        in_maps.append(im)

    import os
    trace = bool(os.environ.get("KERNEL_TRACE"))
    res = run_bass_kernel_spmd(nc, in_maps, core_ids=list(range(8)), trace=trace)
    LAST_EXEC_TIME_NS = res.exec_time_ns
    results = res.results

    out = np.empty((B, S, D), dtype=np.float32)
    for b in range(B):
        out[b] = results[2 * b]["out"] + results[2 * b + 1]["out"]
    if b_O.any():
        out += b_O
    return out


# revision 47
# speedup vs baseline: 1.2578x; 1.0010x over previous
"""Causal multi-head attention on 8 TRN2 NeuronCores.

Problem: x[4, 2048, 768], 12 heads x d_head 64, causal softmax attention.

Sharding: core c handles batch b = c//2 and the 6-head group h0 = 6*(c%2).
Each core computes its partial output o_partial[2048, 768] = sum over its 6
heads of (softmax(QK^T/8) V) @ W_O.  The two cores sharing a batch are summed
on the host (part of unsharding), so the device graph needs no collectives.

Device layout (per core) avoids every on-chip transpose:
  - host passes x^T  [768, 2048]  (xt)
  - Q^T, K^T [384, 2048] = W^T @ x^T   (lhsT = W slice, rhs = x^T)
  - V        [2048, 384]  = x @ W_V    (lhsT = x^T slice, rhs = W_V)
  - scores^T blocks [128k, 512q] = (K^T slice).T @ Q^T slice  (per head, K=64)
  - exp on ACT (scale=1/8 folded in); causal mask = 0/1 multiply on diagonal
    blocks; V is stored augmented with a ones column per head so the single
    AV matmul produces both z^T rows (64) and the softmax denominators (row 64)
  - z^T normalized via DMA-broadcast reciprocal row, stored as ZT [384, 2048]
  - out = (ZT).T @ W_O   (lhsT = ZT slice, rhs = W_O)
All matmuls run in bf16 (1 cycle/column vs 4 for fp32; f32 PSUM accum).

Scheduling: the attention phase is ACT(exp)-bound (~35us of exp per head
pair vs ~25us of PE), while projections and the output projection are
PE-only.  So emission interleaves them: while pair hp's attention stream
keeps ACT busy, the PE also runs pair hp+1's Q/K projections (and, during
the last pair, the output-projection tiles), paced by a credit-fed global
queue with per-sweep deadlines (emission-order WAR safety) and eligibility
gates (fillers reserved for PE-thin phases; out-proj gated on the ZT writes
it reads being emitted).  q-supertiles are processed one at a time (not in
halves) so PSUM fits scores (4 banks) + z accumulators (2) + a
projection/output slot (2).  Every score block writes the two heads'
columns anchored at the psum bank boundary (par0 [q0,512) in bank 0, par1
[512,512+w) in bank 1): the concurrent matmuls hit different single-port
banks, and one exp instruction covers both heads.  Deferred AV / normalize
closures ride a global lag queue that flows across sweep and pair
boundaries; causal-mask multiplies run on the otherwise-idle GpSimd engine;
the last supertile's output projection is split so only the ZT[2] matmul
waits on the final normalize.

Note: the chip drops the PE from 2.4 to ~2.0 GHz (P0 power state) under
sustained load, so back-to-back benchmark runs measure ~10-13% slower than
cooled runs (~236us cooled, ~268us hot).
"""

import sys

if "/opt/trn_rl_repo" not in sys.path:
    sys.path.insert(0, "/opt/trn_rl_repo")

import numpy as np
import ml_dtypes

BF16NP = ml_dtypes.bfloat16


def _ensure_ntff_hook():
    """The agent image's `antenv` lacks `axon_hooks`, which bass_utils needs
    for trace=True under axon. Recreate it via sys.modules injection using the
    boot helper's ctypes wrapper around libaxon_pjrt.so."""
    import types
    if "antenv.axon_hooks" in sys.modules:
        return
    try:
        from trn_agent_boot.trn_boot import _ntff_profile_via_ctypes
        hook = _ntff_profile_via_ctypes("/opt/axon/libaxon_pjrt.so")
    except Exception:
        hook = None
    m = types.ModuleType("antenv.axon_hooks")
    m._hook = hook
    m.get_axon_ntff_profile_hook = lambda: m._hook
    def _set(h):
        m._hook = h
    m.set_axon_ntff_profile_hook = _set
    sys.modules["antenv.axon_hooks"] = m


_ensure_ntff_hook()

import concourse.bass as bass
import concourse.tile as tile
from concourse import bacc, mybir
from concourse.bass_utils import run_bass_kernel_spmd

F32 = mybir.dt.float32
BF16 = mybir.dt.bfloat16
AF = mybir.ActivationFunctionType

D = 768          # d_model
S = 2048         # seq
E = 64           # d_head
NHC = 6          # heads per core
HE = NHC * E     # 384
KD = D // 128    # 6 k-chunks over d_model
B = 4

AV_LAG = 8       # deferred-AV depth (keeps PE from stalling on exp)

LAST_EXEC_TIME_NS = None
_GRAPH_CACHE = {}


def _build_graph(qkv_bias: bool) -> bass.Bass:
    nc = bacc.Bacc("TRN2", target_bir_lowering=False)
    xt = nc.declare_dram_parameter("xt", [D, S], BF16, isOutput=False)
    wq = nc.declare_dram_parameter("wq", [D, HE], BF16, isOutput=False)
    wk = nc.declare_dram_parameter("wk", [D, HE], BF16, isOutput=False)
    wv = nc.declare_dram_parameter("wv", [D, HE], BF16, isOutput=False)
    wo = nc.declare_dram_parameter("wo", [HE, D], BF16, isOutput=False)
    mask = nc.declare_dram_parameter("mask", [128, 128], BF16, isOutput=False)
    if qkv_bias:
        bq = nc.declare_dram_parameter("bq", [HE, 1], F32, isOutput=False)
        bk = nc.declare_dram_parameter("bk", [HE, 1], F32, isOutput=False)
        bv = nc.declare_dram_parameter("bv", [1, HE], BF16, isOutput=False)
    out = nc.declare_dram_parameter("out", [S, D], BF16, isOutput=True)

    with tile.TileContext(nc) as tc:
        with tc.tile_pool(name="persist", bufs=1) as persist, \
             tc.tile_pool(name="workE", bufs=14) as workE, \
             tc.tile_pool(name="workZ", bufs=8) as workZ, \
             tc.tile_pool(name="work2", bufs=4) as work2, \
             tc.tile_pool(name="workO", bufs=4) as workO, \
             tc.tile_pool(name="dramP", bufs=3, space="DRAM") as dramP, \
             tc.tile_pool(name="psP", bufs=2, space="PSUM") as psP, \
             tc.tile_pool(name="psS", bufs=2, space="PSUM") as psS, \
             tc.tile_pool(name="psZ", bufs=1, space="PSUM") as psZ:

            QT = [persist.tile([128, S], BF16, tag=f"qt{m}", name=f"qt{m}") for m in range(3)]
            KT = [persist.tile([128, S], BF16, tag=f"kt{m}", name=f"kt{m}") for m in range(3)]
            ZT = [persist.tile([128, S], BF16, tag=f"zt{m}", name=f"zt{m}") for m in range(3)]
            VA = [persist.tile([128, NHC * 65], BF16, tag=f"va{s}", name=f"va{s}") for s in range(16)]
            WO = [persist.tile([128, D], BF16, tag=f"wo{m}", name=f"wo{m}") for m in range(3)]
            MSK = persist.tile([128, 128], BF16, tag="mask", name="mask_sb")
            XT = [persist.tile([128, S], BF16, tag=f"xt{k}", name=f"xt{k}") for k in range(KD)]
            WQs = [persist.tile([128, HE], BF16, tag=f"wq{k}", name=f"wq{k}") for k in range(KD)]
            WKs = [persist.tile([128, HE], BF16, tag=f"wk{k}", name=f"wk{k}") for k in range(KD)]
            WVs = [persist.tile([128, HE], BF16, tag=f"wv{k}", name=f"wv{k}") for k in range(KD)]

            # issue order matters: the Sync engine issues dma_starts serially
            # (~340ns each), so the first Q/K projection group's inputs
            # (XT[k], WQs[k], WKs[k]) go first, in consumption order.
            for k in range(KD):
                nc.sync.dma_start(out=XT[k][:], in_=xt[k * 128:(k + 1) * 128, :])
                nc.sync.dma_start(out=WQs[k][:], in_=wq[k * 128:(k + 1) * 128, :])
                nc.sync.dma_start(out=WKs[k][:], in_=wk[k * 128:(k + 1) * 128, :])
            for k in range(KD):
                nc.sync.dma_start(out=WVs[k][:], in_=wv[k * 128:(k + 1) * 128, :])
            for m in range(3):
                nc.sync.dma_start(out=WO[m][:], in_=wo[m * 128:(m + 1) * 128, :])
            nc.sync.dma_start(out=MSK[:], in_=mask[:])
            ONES = persist.tile([1, 128], BF16, tag="ones", name="ones_sb")
            nc.vector.memset(ONES[:], 1.0)
            DUM = persist.tile([128, 512], BF16, tag="dum", name="dum_sb")
            nc.vector.memset(DUM[:], 1.0)

            def warm(n):
                # dependency-free matmuls: keep the PE busy (and the HAM
                # clock un-throttled) across stalls the scheduler can't fill
                # -- startup DMA waits and the tail normalize chain.  K must
                # be 128 (full rows): a thin-K stream reads as LOW activity
                # and actively cools the clock.
                for _ in range(n):
                    d = psP.tile([128, 512], F32, tag="psP", name="ps_warm")
                    nc.tensor.matmul(d[:], DUM[:, 0:128], DUM[:],
                                     start=True, stop=True)
            if qkv_bias:
                BQ = persist.tile([128, 3], F32, tag="bq", name="bq_sb")
                BK = persist.tile([128, 3], F32, tag="bk", name="bk_sb")
                BV = persist.tile([1, HE], BF16, tag="bv", name="bv_sb")
                for m in range(3):
                    nc.sync.dma_start(out=BQ[:, m:m + 1], in_=bq[m * 128:(m + 1) * 128, :])
                    nc.sync.dma_start(out=BK[:, m:m + 1], in_=bk[m * 128:(m + 1) * 128, :])
                nc.sync.dma_start(out=BV[:], in_=bv[:])
            for sc in range(16):
                nc.vector.memset(VA[sc][:], 1.0)

            # ---------------- interleave items ----------------
            # Filler work is split to single-matmul granularity (~200ns per
            # item) so feed() can smooth the PE load between attention blocks
            # -- chunky fillers leave micro-gaps that re-throttle the HAM
            # clock.  Items of one accumulation group share the psum tile via
            # a closure dict and stay adjacent in GQ order.
            def qk_items(is_q, m, n):
                st = {}
                items = []

                def mk(k):
                    def go():
                        if k == 0:
                            st["ps"] = psP.tile([128, 512], F32, tag="psP",
                                                name="ps_p")
                        Wt = WQs if is_q else WKs
                        nc.tensor.matmul(
                            st["ps"][:],
                            Wt[k][:, m * 128:(m + 1) * 128],
                            XT[k][:, n * 512:(n + 1) * 512],
                            start=(k == 0), stop=(k == KD - 1))
                    return go
                for k in range(KD):
                    items.append(mk(k))

                def cp():
                    dst = (QT if is_q else KT)[m][:, n * 512:(n + 1) * 512]
                    if qkv_bias:
                        bias_t = BQ if is_q else BK
                        nc.scalar.activation(dst, st["ps"][:], AF.Copy,
                                             bias=bias_t[:, m:m + 1])
                    else:
                        nc.vector.tensor_copy(dst, st["ps"][:])
                items.append(cp)
                return items

            def v_items(sc):
                st = {}
                items = []
                last_mm = KD - 1

                def mk(k):
                    def go():
                        if k == 0:
                            st["ps"] = psP.tile([128, 512], F32, tag="psP",
                                                name="ps_p")
                        nc.tensor.matmul(
                            st["ps"][:, 0:HE],
                            XT[k][:, sc * 128:(sc + 1) * 128],
                            WVs[k][:],
                            start=(k == 0),
                            stop=False if qkv_bias else (k == last_mm))
                    return go
                for k in range(KD):
                    items.append(mk(k))
                if qkv_bias:
                    items.append(lambda: nc.tensor.matmul(
                        st["ps"][:, 0:HE], ONES[:], BV[:],
                        start=False, stop=True))

                def cp():
                    nc.vector.tensor_copy(
                        VA[sc][:].rearrange("p (h c) -> p h c", c=65)[:, :, 0:64],
                        st["ps"][:, 0:HE].rearrange("p (h c) -> p h c", c=64))
                items.append(cp)
                return items

            def out_items(mc, half):
                st = {}
                items = []
                n0 = half * HE

                def mk(k):
                    def go():
                        if k == 0:
                            st["ps"] = psP.tile([128, 512], F32, tag="psP",
                                                name="ps_p")
                        nc.tensor.matmul(
                            st["ps"][:, 0:HE],
                            ZT[k][:, mc * 128:(mc + 1) * 128],
                            WO[k][:, n0:n0 + HE],
                            start=(k == 0), stop=(k == 2))
                    return go
                for k in range(3):
                    items.append(mk(k))

                def cp():
                    ob = workO.tile([128, HE], BF16, tag="ob", name="ob")
                    nc.vector.tensor_copy(ob[:], st["ps"][:, 0:HE])
                    nc.sync.dma_start(
                        out=out[mc * 128:(mc + 1) * 128, n0:n0 + HE],
                        in_=ob[:])
                items.append(cp)
                return items

            def out_split_items(mc, half):
                st = {}
                n0 = half * HE

                def p0():
                    st["ps"] = psP.tile([128, 512], F32, tag="psP",
                                        name="ps_p")
                    nc.tensor.matmul(
                        st["ps"][:, 0:HE],
                        ZT[0][:, mc * 128:(mc + 1) * 128],
                        WO[0][:, n0:n0 + HE], start=True, stop=False)

                def p1():
                    nc.tensor.matmul(
                        st["ps"][:, 0:HE],
                        ZT[1][:, mc * 128:(mc + 1) * 128],
                        WO[1][:, n0:n0 + HE], start=False, stop=True)

                def pcp():
                    st["op"] = workO.tile([128, HE], F32, tag="op01",
                                          name="op01", bufs=8)
                    nc.vector.tensor_copy(st["op"][:], st["ps"][:, 0:HE])

                def f0():
                    # ZT[2]'s last supertile holds RAW z: run the two heads
                    # as concurrent K=64 matmuls (disjoint row groups, two
                    # psum banks) and normalize in the combine below.
                    st["psA"] = psP.tile([128, 512], F32, tag="psP",
                                         name="ps_p")
                    nc.tensor.matmul(
                        st["psA"][:, 0:HE],
                        ZT[2][0:64, mc * 128:(mc + 1) * 128],
                        WO[2][0:64, n0:n0 + HE], start=True, stop=True)
                    st["psB"] = psP.tile([128, 512], F32, tag="psP",
                                         name="ps_p")
                    nc.tensor.matmul(
                        st["psB"][:, 0:HE],
                        ZT[2][64:128, mc * 128:(mc + 1) * 128],
                        WO[2][64:128, n0:n0 + HE], start=True, stop=True)

                def f1():
                    # ob = psA/denomA + psB/denomB + partial01, with the
                    # reciprocals as per-partition scalars (RC4[par][:, cc])
                    cc = mc - 12
                    mult = mybir.AluOpType.mult
                    add = mybir.AluOpType.add
                    ob = workO.tile([128, HE], BF16, tag="ob", name="ob")
                    nc.vector.scalar_tensor_tensor(
                        ob[:], st["psA"][:, 0:HE], RC4[0][:, cc:cc + 1],
                        st["op"][:], mult, add)
                    nc.vector.scalar_tensor_tensor(
                        ob[:], st["psB"][:, 0:HE], RC4[1][:, cc:cc + 1],
                        ob[:], mult, add)
                    nc.sync.dma_start(
                        out=out[mc * 128:(mc + 1) * 128, n0:n0 + HE],
                        in_=ob[:])
                return [p0, p1, pcp], [f0, f1]

            # Global interleave queue: (deadline, pred, fn).
            #  - deadline (hp, t, 0): the item WRITES a tile some score block
            #    of sweep (hp, t) reads, so it must be emitted before that
            #    sweep starts (emission-order WAR hazard otherwise).
            #    Enforced by drain() at every sweep start.
            #  - pred(pos) -> bool: earliest feed() may emit it.  Position
            #    gates reserve fillers for later, PE-thin phases; out-proj
            #    items are gated on the ZT writes they read being emitted
            #    (zt_ready counts normalize part2s per (hp, t)).
            # Q/K chunk n feeds sweep t=n of its pair; V[sc] feeds pair-0
            # sweep sc//4.
            GQ = []
            FAR = (9, 9, 9)
            zt_ready = {}
            RC4 = {}   # par -> [128,4] reciprocal cols for the raw-z tail

            def after(mark):
                return lambda pos: pos >= mark

            def zt_done(tq):
                return lambda pos: zt_ready.get((2, tq), 0) >= 2

            def put(deadline, pred, items):
                # one GQ entry per GROUP: the group's matmuls stay adjacent
                # in the PE stream (splitting them destroys LDWEIGHTS
                # pipelining); credit is spent per-item so pacing stays fine.
                GQ.append((deadline, pred, items, len(items)))

            always = after((-1, -1, -1))

            def sweep_before(m, n):
                # deadline one sweep EARLY: proj tiles must be written well
                # before the first score block that reads them, or the score
                # stalls on the projection's fresh psum->SBUF copy.
                return (m, n - 1, 0) if n >= 1 else (m - 1, 3, 0)

            # pair-0 remainder, ordered V-first within each deadline class
            for n in (2, 3):
                for sc in range(4 * n, 4 * n + 4):
                    put(sweep_before(0, n), always, v_items(sc))
                put(sweep_before(0, n), always, qk_items(True, 0, n))
                put(sweep_before(0, n), always, qk_items(False, 0, n))
            for n in range(4):
                put(sweep_before(1, n), always, qk_items(True, 1, n))
                put(sweep_before(1, n), always, qk_items(False, 1, n))
            # pair-2 projections: n0/n1 reserved for D(1), n2/n3 for D(2)'s
            # first two sweeps (which otherwise have no filler work).
            for n in range(4):
                pred = after((1, 0, 0)) if n < 2 else after((2, 0, 0))
                put(sweep_before(2, n), pred, qk_items(True, 2, n))
                put(sweep_before(2, n), pred, qk_items(False, 2, n))
            for tq in range(3):
                for mc in range(4 * tq, 4 * tq + 4):
                    for half in (0, 1):
                        put(FAR, zt_done(tq), out_items(mc, half))
            # tq=3 split: the ZT[0]/ZT[1] contributions run during sweep 3
            # (partial into SBUF); only the ZT[2] matmul + add + DMA wait for
            # the last normalize, shrinking the tail stall.
            _tq3 = [(mc, half) for mc in range(12, 16) for half in (0, 1)]
            _splits = {k: out_split_items(*k) for k in _tq3}
            for k in _tq3:
                put(FAR, after((2, 3, 0)), _splits[k][0])
            for k in _tq3:
                put(FAR, zt_done(3), _splits[k][1])

            feed_credit = [0.0]

            def run_group(entry):
                for it in entry[2]:
                    it()

            def feed(pos, r):
                feed_credit[0] += r
                while GQ and feed_credit[0] >= GQ[0][3] and GQ[0][1](pos):
                    e = GQ.pop(0)
                    run_group(e)
                    feed_credit[0] -= e[3]

            def drain(pos):
                # deadline-forced emission (sweep-start prerequisites)
                while GQ and GQ[0][0] <= pos:
                    run_group(GQ.pop(0))

            def drain_all():
                while GQ:
                    run_group(GQ.pop(0))

            # ---------------- attention stream ----------------
            # av_q is GLOBAL: the deferred-AV lag flows across pair
            # boundaries, so a new pair's (independent) score blocks and
            # sweep-start filler bursts run while the old pair's last exps
            # finish, instead of the PE head-of-line blocking on them.
            av_q = []      # aged deferred work: (fn, norm_fn | None)

            def pump_avs(lag):
                while len(av_q) > lag:
                    av_fn, norm_fn = av_q.pop(0)
                    av_fn()
                    if norm_fn is not None:
                        norm_fn()

            def run_pair(hp, rate):
                psz = {}

                def emit_normalize(par, t):
                    ho = par * 64
                    # drain psz out of PSUM fast (frees the bank)
                    zraw = workZ.tile([65, 512], BF16, tag="zraw", name="zraw")
                    nc.vector.tensor_copy(zraw[:], psz[par][:])
                    zr = zraw[64:65, :]

                    if hp == 2 and t == 3:
                        # Final supertile: skip the broadcast-normalize whose
                        # DMA latency would sit exposed at the kernel tail.
                        # ZT gets RAW z; the reciprocal lands as a [128, 4]
                        # per-partition column (partition = q-local, column =
                        # mc chunk) and is applied as a tensor_scalar operand
                        # in the output combine, which overlaps the bounce.
                        dd = dramP.tile([1, 512], BF16, tag=f"dd{par}",
                                        name=f"dd{par}")
                        nc.sync.dma_start(out=dd[:], in_=zr)
                        dd_ap = dd[:]
                        rp4 = work2.tile([128, 4], BF16, tag=f"rp4{par}",
                                         name=f"rp4{par}")
                        nc.sync.dma_start(out=rp4[:], in_=bass.AP(
                            tensor=dd_ap.tensor, offset=dd_ap.offset,
                            ap=[[1, 128], [128, 4]]))
                        rc4 = work2.tile([128, 4], BF16, tag=f"rc4{par}",
                                         name=f"rc4{par}")
                        with nc.allow_low_precision(reason="softmax recip"):
                            nc.vector.reciprocal(rc4[:], rp4[:])
                        RC4[par] = rc4

                        def part2r():
                            nc.vector.tensor_copy(
                                ZT[hp][ho:ho + 64, t * 512:(t + 1) * 512],
                                zraw[0:64, :])
                            zt_ready[(hp, t)] = zt_ready.get((hp, t), 0) + 1
                        av_q.append((part2r, None))
                        return

                    # reciprocal of the denominator row, reshaped across all
                    # 128 DVE lanes via a direct SBUF->SBUF strided DMA
                    # ([1,512] row -> [128,4]); a 1-lane [1,512] reciprocal
                    # costs 3.3us.
                    rp = work2.tile([128, 4], BF16, tag="rp", name="rp")
                    nc.sync.dma_start(out=rp[:], in_=bass.AP(
                        tensor=zr.tensor, offset=zr.offset,
                        ap=[zr.ap[0], [4, 128], [1, 4]]))
                    rcp = work2.tile([128, 4], BF16, tag="rcp", name="rcp")
                    with nc.allow_low_precision(reason="softmax recip bf16"):
                        nc.vector.reciprocal(rcp[:], rp[:])
                    rcd = dramP.tile([1, 512], BF16, tag="rcd", name="rcd")
                    rcd_ap = rcd[:]
                    nc.sync.dma_start(out=bass.AP(
                        tensor=rcd_ap.tensor, offset=rcd_ap.offset,
                        ap=[[4, 128], [1, 4]]), in_=rcp[:])
                    bc = work2.tile([64, 512], BF16, tag="bc", name="bc")
                    nc.sync.dma_start(out=bc[:], in_=bass.AP(
                        tensor=rcd_ap.tensor, offset=rcd_ap.offset,
                        ap=[[0, 64], rcd_ap.ap[-1]]))

                    def part2():
                        # final scale, re-queued behind AV_LAG more items so
                        # the reciprocal's DMA-bounce latency is hidden
                        nc.vector.tensor_mul(
                            ZT[hp][ho:ho + 64, t * 512:(t + 1) * 512],
                            zraw[0:64, :], bc[:])
                        zt_ready[(hp, t)] = zt_ready.get((hp, t), 0) + 1
                    av_q.append((part2, None))

                for t in range(4):
                    drain((hp, t, 0))
                    for j in range(4 * t + 4):
                        r = j - 4 * t  # >= 0 only on the diagonal
                        q0 = 128 * r if r >= 0 else 0
                        # Both heads' scores in ONE psum tile, anchored at the
                        # bank boundary: par0 writes [q0, 512) (bank 0), par1
                        # [512, 512+w) (bank 1).  The two matmuls run
                        # concurrently (disjoint row groups) so they must hit
                        # DIFFERENT single-port psum banks, and the regions
                        # stay contiguous so one exp instruction covers both.
                        w = 512 - q0
                        c0s = (q0, 512)
                        pss = psS.tile([128, 1024], F32, tag="pss", name="pss")
                        for par in (0, 1):
                            ho = par * 64
                            nc.tensor.matmul(
                                pss[:, c0s[par]:c0s[par] + w],
                                KT[hp][ho:ho + 64, j * 128:(j + 1) * 128],
                                QT[hp][ho:ho + 64, t * 512 + q0:(t + 1) * 512],
                                start=True, stop=True)
                        et = workE.tile([128, 1024], BF16, tag="et", name="et")
                        nc.scalar.activation(et[:, q0:512 + w],
                                             pss[:, q0:512 + w],
                                             AF.Exp, scale=0.125)
                        if r >= 0:
                            # gpsimd (otherwise idle): keeps the DVE queue
                            # short so AV-gating work isn't behind proj copies
                            for c0 in c0s:
                                nc.gpsimd.tensor_mul(
                                    et[:, c0:c0 + 128],
                                    et[:, c0:c0 + 128],
                                    MSK[:])
                        for par in (0, 1):
                            a0 = c0s[par]  # valid region start in et

                            def av_fn(par=par, et=et, j=j, t=t, a0=a0, q0=q0):
                                if j == 0:
                                    psz[par] = psZ.tile(
                                        [65, 512], F32, tag=f"psz{par}",
                                        name=f"psz{par}")
                                h = 2 * hp + par
                                nc.tensor.matmul(
                                    psz[par][:, q0:512],
                                    VA[j][:, h * 65:(h + 1) * 65],
                                    et[:, a0:a0 + 512 - q0],
                                    start=(j == 0), stop=(j == 4 * t + 3))
                            norm_fn = (
                                lambda par=par, t=t:
                                emit_normalize(par, t)) if j == 4 * t + 3 else None
                            av_q.append((av_fn, norm_fn))
                        feed((hp, t, j), rate)
                        pump_avs(AV_LAG)
                    pump_avs(2)

            # upfront projections for pair 0's first two sweeps (PE-only
            # prologue; everything else interleaves into the D streams).
            for n in (0, 1):
                for it in qk_items(True, 0, n) + qk_items(False, 0, n):
                    it()
            for sc in range(8):
                for it in v_items(sc):
                    it()

            run_pair(0, 2.6)
            run_pair(1, 2.2)
            run_pair(2, 3.2)
            pump_avs(0)
            drain_all()
    nc.compile()
    return nc


def _build_mask() -> np.ndarray:
    # triangle for the strict-diagonal 128x128 strip: 1.0 iff q_local >= k_local
    kl = np.arange(128)[:, None]
    ql = np.arange(128)[None, :]
    return (ql >= kl).astype(np.float32)


def kernel(**inputs) -> np.ndarray:
    global LAST_EXEC_TIME_NS
    x = np.asarray(inputs["normalized_resid_pre"], dtype=np.float32)
    W_Q = np.asarray(inputs["W_Q"], dtype=np.float32)
    W_K = np.asarray(inputs["W_K"], dtype=np.float32)
    W_V = np.asarray(inputs["W_V"], dtype=np.float32)
    W_O = np.asarray(inputs["W_O"], dtype=np.float32)
    b_Q = np.asarray(inputs["b_Q"], dtype=np.float32)
    b_K = np.asarray(inputs["b_K"], dtype=np.float32)
    b_V = np.asarray(inputs["b_V"], dtype=np.float32)
    b_O = np.asarray(inputs["b_O"], dtype=np.float32)

    qkv_bias = bool(b_Q.any() or b_K.any() or b_V.any())
    key = qkv_bias
    if key not in _GRAPH_CACHE:
        _GRAPH_CACHE[key] = _build_graph(qkv_bias)
    nc = _GRAPH_CACHE[key]

    mask = _build_mask()
    in_maps = []
    for c in range(8):
        b, h0 = c // 2, NHC * (c % 2)
        im = {
            "xt": np.ascontiguousarray(x[b].T).astype(BF16NP),
            "wq": np.ascontiguousarray(
                W_Q[h0:h0 + NHC].transpose(1, 0, 2).reshape(D, HE)).astype(BF16NP),
            "wk": np.ascontiguousarray(
                W_K[h0:h0 + NHC].transpose(1, 0, 2).reshape(D, HE)).astype(BF16NP),
            "wv": np.ascontiguousarray(
                W_V[h0:h0 + NHC].transpose(1, 0, 2).reshape(D, HE)).astype(BF16NP),
            "wo": np.ascontiguousarray(W_O[h0:h0 + NHC].reshape(HE, D)).astype(BF16NP),
            "mask": mask.astype(BF16NP),
        }
        if qkv_bias:
            im["bq"] = np.ascontiguousarray(b_Q[h0:h0 + NHC].reshape(HE, 1))
            im["bk"] = np.ascontiguousarray(b_K[h0:h0 + NHC].reshape(HE, 1))
            im["bv"] = np.ascontiguousarray(b_V[h0:h0 + NHC].reshape(1, HE)).astype(BF16NP)
        in_maps.append(im)

    import os
    trace = bool(os.environ.get("KERNEL_TRACE"))
    res = run_bass_kernel_spmd(nc, in_maps, core_ids=list(range(8)), trace=trace)
    LAST_EXEC_TIME_NS = res.exec_time_ns
    results = res.results

    out = np.empty((B, S, D), dtype=np.float32)
    for b in range(B):
        out[b] = (np.asarray(results[2 * b]["out"], dtype=np.float32)
                  + np.asarray(results[2 * b + 1]["out"], dtype=np.float32))
    if b_O.any():
        out += b_O
    return out
